# revision 15
# baseline (speedup 1.0000x reference)
"""DiffusionGraphConv on 8 Trainium2 NeuronCores (Bass/Tile).

out = sum_k (D^-1 A)^k x W_f[k] + ((D^-1 A)^T)^k x W_b[k] + bias, K=2,
N=50000 nodes, E=800000 edges, B=8, C_in=C_out=64, f32.

Sharding: 8 cores = 2 diffusion directions x 2 batch-groups (4 batches
packed per 512B bf16 gather token) x 2 node-halves. No cross-core
traffic: hop 1 processes edges whose DESTINATION block falls in the
core's half (gathering from the replicated x), producing that half of
h1 = (D^-1 A) h0; hop 2 processes edges whose SOURCE falls in the same
half (gathering only from the core's own h1) and scatter-adds into all
destination blocks. The four partial outputs per batch-group (2 dirs x
2 halves) are summed on the host together with the bias.

Per hop on device: messages h[src[e]] are fetched with nc.gpsimd.dma_gather
(512B bf16 tokens); the scatter-add is a TensorE matmul per 128-edge chunk
with a one-hot matrix S[t,n] = (n == dst_local[t]) * nv[t] built by one DVE
tensor_scalar(is_equal, mult) op in bf16 (4x DVE mode); chunks accumulate
per 128-row node block in PSUM. Hop 2 accumulates the transposed block
(lhsT=chunk) directly so no PE transpose is needed before the W matmul.

Blocks are assigned to program positions per-core by descending chunk
count (bin-packing) so one SPMD program's per-position chunk counts,
taken as the max over the 4 edge-shard variants, waste little padding.
"""
import numpy as np
import ml_dtypes

import concourse.bacc as bacc
import concourse.tile as tile
import concourse.mybir as mybir
from concourse.bass_utils import run_bass_kernel_spmd
from concourse.masks import make_identity

P = 128
N_NODES = 50000
N_EDGES = 800000
B, C = 8, 64
NNP = 50048          # nodes padded to a multiple of 128
NB = NNP // P        # 391 destination blocks
NPOS1 = 196          # hop-1 program positions (half0: 196 blocks, half1: 195)
HALF_ROWS = NPOS1 * P   # 25088 rows of h1 per core
LO_LIMIT = 32768     # src < LO_LIMIT -> lo gather stream (int16 idx range)
HI_BASE = NNP - 32768   # hi stream gathers rows [HI_BASE:], idx = src - HI_BASE
GATHER_SLAB = 4096   # tokens per dma_gather instruction
TOKC = 4 * C         # 256 bf16 values per token (4 batches x 64 ch) = 512B
dt = mybir.dt
bf16 = ml_dtypes.bfloat16

BUFS = dict(msg_lo=3, msg_hi=2, msg_lo2=3, idxp=8, spp=12, blkp=5,
            psh=3, pstr=2, psout=2)

_prog_cache = {}


# ---------------- host-side prep ----------------

def _classify(pos, src, npos):
    """Per-position (must-lo, must-hi, flexible) source counts."""
    ml = np.bincount(pos[src < HI_BASE], minlength=npos)
    mh = np.bincount(pos[src >= LO_LIMIT], minlength=npos)
    fx = np.bincount(pos[(src >= HI_BASE) & (src < LO_LIMIT)], minlength=npos)
    return ml, mh, fx


def _choose_chunks(cands, npos):
    """Unified per-position (L, H) chunk counts covering every variant in
    `cands` (list of (ml, mh, fx) triples), minimizing L+H; plus each
    variant's flex-to-lo counts."""
    tots = [ml + mh + fx for (ml, mh, fx) in cands]
    L = np.zeros(npos, np.int64)
    H = np.zeros(npos, np.int64)
    for p in range(npos):
        lmin = max((int(ml[p]) + P - 1) // P for (ml, _, _) in cands)
        lmax = min(int(ml[p] + fx[p]) // P for (ml, _, fx) in cands)
        lfull = max((int(t[p]) + P - 1) // P for t in tots)
        best = None
        for Lc in range(lmin, max(lmin, lmax, lfull) + 2):
            need = 0
            for (ml, mh, fx), tot in zip(cands, tots):
                rem = int(tot[p]) - min(Lc * P, int(ml[p] + fx[p]))
                need = max(need, rem, int(mh[p]))
            Hc = (need + P - 1) // P
            # <= so ties prefer the larger lo stream (keeps hop-2 hi empty)
            if best is None or Lc + Hc <= best[0] + best[1]:
                best = (Lc, Hc)
        L[p], H[p] = best
    L[(L + H) == 0] = 1   # keep every position's PSUM block defined
    f2l = [np.minimum(L * P, ml + fx) - ml for (ml, _, fx) in cands]
    return L, H, f2l


def _build_stream(pos, dstloc, src, nv, L, H, flex_to_lo, npos):
    """Padded token streams + chunk-major meta for one shard variant.

    Position p's lo tokens occupy lo-stream slots [cumsum, +L[p]*128), hi
    tokens likewise. Device chunk c = chunk_off[p]+j uses lo chunk j for
    j < L[p], else hi chunk j-L[p]. Padding tokens: idx 0 / nv 0 / dst 0.
    """
    lo_chunk_off = np.concatenate([[0], np.cumsum(L)[:-1]])
    hi_chunk_off = np.concatenate([[0], np.cumsum(H)[:-1]])
    chunk_off = np.concatenate([[0], np.cumsum(L + H)[:-1]])
    NCH = int((L + H).sum())
    TLO, THI = int(L.sum()) * P, int(H.sum()) * P

    lo = src < HI_BASE
    flex = (src >= HI_BASE) & (src < LO_LIMIT)
    fidx = np.flatnonzero(flex)
    forder = np.argsort(pos[fidx], kind="stable")
    fpos = pos[fidx[forder]]
    fcnt = np.bincount(fpos, minlength=npos)
    fstart = np.concatenate([[0], np.cumsum(fcnt)[:-1]])
    frank = np.arange(fidx.size) - fstart[fpos]
    lo = lo.copy()
    lo[fidx[forder]] = frank < flex_to_lo[fpos]
    assert (np.bincount(pos[lo], minlength=npos) <= L * P).all()
    assert (np.bincount(pos[~lo], minlength=npos) <= H * P).all()
    order = np.lexsort((~lo, pos))
    dl_s, s_s, nv_s = dstloc[order], src[order], nv[order]
    pos_s, lo_s = pos[order], lo[order]
    gid = pos_s * 2 + (~lo_s).astype(np.int64)
    cnt = np.bincount(gid, minlength=npos * 2)
    gstart = np.concatenate([[0], np.cumsum(cnt)[:-1]])
    rank = np.arange(dl_s.size) - gstart[gid]
    slot = np.where(lo_s, lo_chunk_off[pos_s] * P + rank,
                    hi_chunk_off[pos_s] * P + rank)

    idx_lo = np.zeros(TLO, np.int16)
    nv_lo = np.zeros(TLO, np.float32)
    rm_lo = np.zeros(TLO, np.float32)
    idx_hi = np.zeros(THI, np.int16)
    nv_hi = np.zeros(THI, np.float32)
    rm_hi = np.zeros(THI, np.float32)
    m = lo_s
    idx_lo[slot[m]] = s_s[m].astype(np.int16)
    nv_lo[slot[m]] = nv_s[m]
    rm_lo[slot[m]] = dl_s[m].astype(np.float32)
    m = ~lo_s
    idx_hi[slot[m]] = (s_s[m] - HI_BASE).astype(np.int16)
    nv_hi[slot[m]] = nv_s[m]
    rm_hi[slot[m]] = dl_s[m].astype(np.float32)

    rowm = np.zeros((P, NCH), np.float32)
    nvm = np.zeros((P, NCH), np.float32)
    lo_cols = (np.repeat(chunk_off, L) +
               (np.arange(TLO // P) - np.repeat(lo_chunk_off, L))) if TLO else []
    hi_cols = (np.repeat(chunk_off + L, H) +
               (np.arange(THI // P) - np.repeat(hi_chunk_off, H))) if THI else []
    if TLO:
        rowm[:, lo_cols] = rm_lo.reshape(-1, P).T
        nvm[:, lo_cols] = nv_lo.reshape(-1, P).T
    if THI:
        rowm[:, hi_cols] = rm_hi.reshape(-1, P).T
        nvm[:, hi_cols] = nv_hi.reshape(-1, P).T

    def wrap(a):  # [T] -> [128, T/16]; token i at [i%16, i//16], replicated 8x
        if a.size == 0:
            return np.zeros((P, 0), np.int16)
        return np.ascontiguousarray(np.tile(a.reshape(a.size // 16, 16).T, (8, 1)))

    return wrap(idx_lo), wrap(idx_hi), rowm, nvm


def _pack_positions(tot, blocks, npos):
    """Assign `blocks` to program positions by descending token count.
    Returns (order, inv) where order[p] = absolute block (-1 pad) and
    inv[blk] = position."""
    o = blocks[np.argsort(-tot[blocks], kind="stable")]
    order = np.full(npos, -1, np.int64)
    order[:o.size] = o
    inv = np.full(NB, -1, np.int64)
    inv[o] = np.arange(o.size)
    return order, inv


# ---------------- device program (SPMD over the 8 cores) ----------------

def _build_program(L1, H1, L2):
    NCH1 = int((L1 + H1).sum())
    NCH2 = int(L2.sum())
    TLO1, THI1 = int(L1.sum()) * P, int(H1.sum()) * P
    T2 = NCH2 * P
    nc = bacc.Bacc("TRN2", target_bir_lowering=False, debug=False, num_devices=1)
    x2 = nc.dram_tensor("x2", [NNP, TOKC], dt.bfloat16, kind="ExternalInput")
    w2_d = nc.dram_tensor("w2", [P, 2, P], dt.bfloat16, kind="ExternalInput")
    idx_d = {
        'lo': nc.dram_tensor("idx_lo", [P, TLO1 // 16], dt.int16, kind="ExternalInput"),
        'hi': nc.dram_tensor("idx_hi", [P, THI1 // 16], dt.int16, kind="ExternalInput"),
        'lo2': nc.dram_tensor("idx2", [P, T2 // 16], dt.int16, kind="ExternalInput"),
    }
    rowm1_d = nc.dram_tensor("rowm1", [P, NCH1], dt.float32, kind="ExternalInput")
    nvm1_d = nc.dram_tensor("nvm1", [P, NCH1], dt.float32, kind="ExternalInput")
    rowm2_d = nc.dram_tensor("rowm2", [P, NCH2], dt.float32, kind="ExternalInput")
    nvm2_d = nc.dram_tensor("nvm2", [P, NCH2], dt.float32, kind="ExternalInput")
    z2 = nc.dram_tensor("z2", [HALF_ROWS, TOKC], dt.bfloat16)
    outA = nc.dram_tensor("outA", [HALF_ROWS, TOKC], dt.bfloat16, kind="ExternalOutput")
    outB = nc.dram_tensor("outB", [NNP, TOKC], dt.bfloat16, kind="ExternalOutput")

    with tile.TileContext(nc) as tc:
        with (tc.tile_pool(name="const", bufs=1) as constp,
              tc.tile_pool(name="meta", bufs=1) as metap,
              tc.tile_pool(name="msg_lo", bufs=BUFS["msg_lo"]) as msglop,
              tc.tile_pool(name="msg_hi", bufs=BUFS["msg_hi"]) as msghip,
              tc.tile_pool(name="msg_lo2", bufs=BUFS["msg_lo2"]) as msglo2p,
              tc.tile_pool(name="idxp", bufs=BUFS["idxp"]) as idxp,
              tc.tile_pool(name="spp", bufs=BUFS["spp"]) as spp,
              tc.tile_pool(name="blkp", bufs=BUFS["blkp"]) as blkp,
              tc.tile_pool(name="psh", bufs=BUFS["psh"], space="PSUM") as psum_h,
              tc.tile_pool(name="pstr", bufs=BUFS["pstr"], space="PSUM") as psum_tr,
              tc.tile_pool(name="psout", bufs=BUFS["psout"], space="PSUM") as psum_out):

            iota_i = constp.tile([P, P], dt.int32)
            nc.gpsimd.iota(iota_i[:], pattern=[[1, P]], base=0, channel_multiplier=0)
            iota_f = constp.tile([P, P], dt.bfloat16)
            nc.vector.tensor_copy(iota_f[:], iota_i[:])
            ident = constp.tile([P, P], dt.bfloat16)
            make_identity(nc, ident[:])
            w2_sb = constp.tile([P, 2, P], dt.bfloat16)
            nc.sync.dma_start(out=w2_sb[:], in_=w2_d[:])
            rowm1_sb = metap.tile([P, NCH1], dt.float32)
            nc.sync.dma_start(out=rowm1_sb[:], in_=rowm1_d[:])
            nvm1_sb = metap.tile([P, NCH1], dt.float32)
            nc.sync.dma_start(out=nvm1_sb[:], in_=nvm1_d[:])
            rowm2_sb = metap.tile([P, NCH2], dt.float32)
            nc.sync.dma_start(out=rowm2_sb[:], in_=rowm2_d[:])
            nvm2_sb = metap.tile([P, NCH2], dt.float32)
            nc.sync.dma_start(out=nvm2_sb[:], in_=nvm2_d[:])

            slab_cache = {}

            def get_chunk(stream, src_ap, pool, T, gpos):
                tile_obj, s_cur = slab_cache.get(stream, (None, -1))
                s, j = divmod(gpos, GATHER_SLAB // P)
                if s != s_cur:
                    off = s * GATHER_SLAB
                    g = min(GATHER_SLAB, T - off)
                    it = idxp.tile([P, g // 16], dt.int16, tag="idx")
                    nc.sync.dma_start(
                        out=it[:], in_=idx_d[stream][:, off // 16:(off + g) // 16])
                    mt = pool.tile([P, g // P, TOKC], dt.bfloat16, tag="m" + stream)
                    nc.gpsimd.dma_gather(
                        out_ap=mt[:], in_ap=src_ap,
                        idxs_ap=it[:], num_idxs=g, num_idxs_reg=g,
                        elem_size=TOKC, single_packet=False)
                    slab_cache[stream] = (mt, s)
                    tile_obj = mt
                return tile_obj, j

            def build_sp(rowm_sb, nvm_sb, c):
                sp = spp.tile([P, P], dt.bfloat16, tag="sp")
                nc.vector.tensor_scalar(
                    sp[:], iota_f[:],
                    rowm_sb[:, c:c + 1], nvm_sb[:, c:c + 1],
                    mybir.AluOpType.is_equal, mybir.AluOpType.mult)
                return sp

            # ---- hop 1: h1[half] = (D^-1 A) h0;  outA = h1 @ W[0] and
            # z2 = h1 @ W[1] (W commutes with A: out = h1 W0 + A (h1 W1),
            # so hop 2 needs no W stage at all). ----
            c = 0
            glo = 0
            ghi = 0
            for p in range(NPOS1):
                Lp, Hp = int(L1[p]), int(H1[p])
                CPB = Lp + Hp
                hp = psum_h.tile([P, 2, P], dt.float32, tag="hp")
                for j in range(CPB):
                    if j < Lp:
                        mt, jj = get_chunk('lo', x2[0:LO_LIMIT, :], msglop,
                                           TLO1, glo + j)
                    else:
                        mt, jj = get_chunk('hi', x2[HI_BASE:NNP, :], msghip,
                                           THI1, ghi + (j - Lp))
                    sp = build_sp(rowm1_sb, nvm1_sb, c + j)
                    nc.tensor.matmul(hp[:], sp[:], mt[:, jj, :],
                                     start=(j == 0), stop=(j == CPB - 1))
                c += CPB
                glo += Lp
                ghi += Hp
                h_sb = blkp.tile([P, 2, P], dt.bfloat16, tag="h_sb")
                nc.scalar.copy(h_sb[:], hp[:])
                tr = psum_tr.tile([P, 2, P], dt.bfloat16, tag="tr")
                nc.tensor.transpose(tr[:, 0, :], h_sb[:, 0, :], ident[:])
                nc.tensor.transpose(tr[:, 1, :], h_sb[:, 1, :], ident[:])
                trs = blkp.tile([P, 2, P], dt.bfloat16, tag="trs")
                nc.vector.tensor_copy(trs[:], tr[:])
                for k, dest in ((1, z2), (0, outA)):
                    op = psum_out.tile([P, 2, P], dt.float32, tag="op")
                    nc.tensor.matmul(op[:, 0, :], trs[:, 0, :], w2_sb[:, k, :],
                                     start=True, stop=True)
                    nc.tensor.matmul(op[:, 1, :], trs[:, 1, :], w2_sb[:, k, :],
                                     start=True, stop=True)
                    ob = blkp.tile([P, 2, P], dt.bfloat16, tag="ob")
                    if k == 1:
                        nc.scalar.copy(ob[:], op[:])
                    else:
                        nc.vector.tensor_copy(ob[:], op[:])
                    nc.sync.dma_start(out=dest[p * P:(p + 1) * P, :], in_=ob[:])

            # ---- hop 2: outB = (D^-1 A)|src-half z2 (final partial) ----
            # The barrier orders hop-2's z2 gathers after hop-1's z2 writes
            # (DRAM RAW is not tracked at tile granularity).
            tc.strict_bb_all_engine_barrier()
            c = 0
            for p in range(NB):
                CPB = int(L2[p])
                hp = psum_h.tile([P, 2, P], dt.float32, tag="hp")
                for j in range(CPB):
                    mt, jj = get_chunk('lo2', z2[0:HALF_ROWS, :], msglo2p, T2,
                                       c + j)
                    sp = build_sp(rowm2_sb, nvm2_sb, c + j)
                    nc.tensor.matmul(hp[:], sp[:], mt[:, jj, :],
                                     start=(j == 0), stop=(j == CPB - 1))
                c += CPB
                ob = blkp.tile([P, 2, P], dt.bfloat16, tag="ob")
                nc.scalar.copy(ob[:], hp[:])
                nc.sync.dma_start(out=outB[p * P:(p + 1) * P, :], in_=ob[:])

    nc.compile()
    return nc


# ---------------- entry point ----------------

def kernel(x, edge_index, edge_vals, W_f, W_b, bias):
    x = np.asarray(x, dtype=np.float32)
    edge_index = np.asarray(edge_index)
    edge_vals = np.asarray(edge_vals, dtype=np.float32)
    W_f = np.asarray(W_f, dtype=np.float32)
    W_b = np.asarray(W_b, dtype=np.float32)
    bias = np.asarray(bias, dtype=np.float32)

    rows = edge_index[0].astype(np.int64)
    cols = edge_index[1].astype(np.int64)
    deg = np.zeros(N_NODES, np.float32)
    np.add.at(deg, rows, edge_vals)
    deg += np.float32(1e-8)
    nv = (edge_vals / deg[rows]).astype(np.float32)

    halves = [np.arange(0, NPOS1), np.arange(NPOS1, NB)]
    v1 = []   # hop-1 variants: (pos, dstloc, src, nv, order)
    v2 = []   # hop-2 variants: (pos, dstloc, srcloc, nv, order, inv1)
    for d in range(2):
        dst, src = (rows, cols) if d == 0 else (cols, rows)
        dblk = dst >> 7
        dloc = dst & (P - 1)
        sblk = src >> 7
        tot1 = np.bincount(dblk, minlength=NB)
        tot2 = np.bincount(dblk, weights=(sblk >= NPOS1).astype(np.float64),
                           minlength=NB)
        for h in range(2):
            sel = (dblk >= NPOS1) == (h == 1)
            order1, inv1 = _pack_positions(tot1, halves[h], NPOS1)
            v1.append((inv1[dblk[sel]], dloc[sel], src[sel], nv[sel], order1))
            sel2 = (sblk >= NPOS1) == (h == 1)
            t2 = tot2 if h == 1 else (tot1 - tot2)
            order2, inv2 = _pack_positions(t2, np.arange(NB), NB)
            srcloc = inv1[sblk[sel2]] * P + (src[sel2] & (P - 1))
            v2.append((inv2[dblk[sel2]], dloc[sel2], srcloc, nv[sel2],
                       order2, inv1))

    c1 = [_classify(pos, src, NPOS1) for (pos, _, src, _, _) in v1]
    L1, H1, f2l1 = _choose_chunks(c1, NPOS1)
    c2 = [_classify(pos, src, NB) for (pos, _, src, _, _, _) in v2]
    L2, H2, f2l2 = _choose_chunks(c2, NB)
    assert H2.sum() == 0, "hop-2 sources must fit the lo stream"

    s1 = [_build_stream(pos, dl, src, nvv, L1, H1, f2l1[i], NPOS1)
          for i, (pos, dl, src, nvv, _) in enumerate(v1)]
    s2 = [_build_stream(pos, dl, src, nvv, L2, H2, f2l2[i], NB)
          for i, (pos, dl, src, nvv, _, _) in enumerate(v2)]

    key = (L1.tobytes(), H1.tobytes(), L2.tobytes())
    if key not in _prog_cache:
        _prog_cache.clear()
        _prog_cache[key] = _build_program(L1, H1, L2)
    nc = _prog_cache[key]

    in_maps = []
    for core in range(8):
        d, g, h = core >> 2, (core >> 1) & 1, core & 1
        vi = d * 2 + h
        Wd = W_f if d == 0 else W_b
        x2 = np.zeros((NNP, TOKC), bf16)
        x2[:N_NODES] = x[4 * g:4 * g + 4].transpose(1, 0, 2).reshape(
            N_NODES, TOKC).astype(bf16)
        w2 = np.zeros((P, 2, P), bf16)
        for k in range(2):
            for a in range(2):
                w2[C * a:C * a + C, k, C * a:C * a + C] = Wd[k].astype(bf16)
        in_maps.append({
            "x2": x2, "w2": w2,
            "idx_lo": s1[vi][0], "idx_hi": s1[vi][1],
            "rowm1": s1[vi][2], "nvm1": s1[vi][3],
            "idx2": s2[vi][0],
            "rowm2": s2[vi][2], "nvm2": s2[vi][3],
        })

    results = run_bass_kernel_spmd(nc, in_maps, list(range(8))).results

    out = np.empty((B, N_NODES, C), np.float32)
    for g in range(2):
        acc = np.zeros((NNP, TOKC), np.float32)
        for d in range(2):
            for h in range(2):
                vi = d * 2 + h
                r = results[(d << 2) | (g << 1) | h]
                order2 = v2[vi][4]
                inv2 = np.argsort(order2)
                acc += np.asarray(r["outB"]).astype(np.float32).reshape(
                    NB, P, TOKC)[inv2].reshape(NNP, TOKC)
                order1 = v1[vi][4]
                nreal = halves[h].size
                oa = np.asarray(r["outA"]).astype(np.float32).reshape(
                    NPOS1, P, TOKC)[:nreal]
                accb = acc.reshape(NB, P, TOKC)
                accb[order1[:nreal]] += oa
        for bl in range(4):
            out[4 * g + bl] = acc[:N_NODES, C * bl:C * bl + C]
    out += bias.reshape(1, 1, C)
    return out


# revision 16
# speedup vs baseline: 1.1109x; 1.1109x over previous
"""DiffusionGraphConv on 8 Trainium2 NeuronCores (Bass/Tile).

out = sum_k (D^-1 A)^k x W_f[k] + ((D^-1 A)^T)^k x W_b[k] + bias, K=2,
N=50000 nodes, E=800000 edges, B=8, C_in=C_out=64, f32.

Sharding: 8 cores = 2 diffusion directions x 2 batch-groups (4 batches
packed per 512B bf16 gather token) x 2 node-halves. No cross-core
traffic: hop 1 processes edges whose DESTINATION block falls in the
core's half (gathering from the replicated x), producing that half of
h1 = (D^-1 A) h0; hop 2 processes edges whose SOURCE falls in the same
half (gathering only from the core's own h1) and scatter-adds into all
destination blocks. The four partial outputs per batch-group (2 dirs x
2 halves) are summed on the host together with the bias.

Per hop on device: messages h[src[e]] are fetched with nc.gpsimd.dma_gather
(512B bf16 tokens); the scatter-add is a TensorE matmul per 128-edge chunk
with a one-hot matrix S[t,n] = (n == dst_local[t]) * nv[t] built by one DVE
tensor_scalar(is_equal, mult) op in bf16 (4x DVE mode); chunks accumulate
per 128-row node block in PSUM. Hop 2 accumulates the transposed block
(lhsT=chunk) directly so no PE transpose is needed before the W matmul.

Blocks are assigned to program positions per-core by descending chunk
count (bin-packing) so one SPMD program's per-position chunk counts,
taken as the max over the 4 edge-shard variants, waste little padding.
"""
import numpy as np
import ml_dtypes

import concourse.bacc as bacc
import concourse.tile as tile
import concourse.mybir as mybir
from concourse.bass_utils import run_bass_kernel_spmd
from concourse.masks import make_identity

P = 128
N_NODES = 50000
N_EDGES = 800000
B, C = 8, 64
NNP = 50048          # nodes padded to a multiple of 128
NB = NNP // P        # 391 destination blocks
NPOS1 = 196          # hop-1 program positions (half0: 196 blocks, half1: 195)
HALF_ROWS = NPOS1 * P   # 25088 rows of h1 per core
LO_LIMIT = 32768     # src < LO_LIMIT -> lo gather stream (int16 idx range)
HI_BASE = NNP - 32768   # hi stream gathers rows [HI_BASE:], idx = src - HI_BASE
GATHER_SLAB = 2048   # tokens per dma_gather instruction
TOKC = 4 * C         # 256 bf16 values per token (4 batches x 64 ch) = 512B
dt = mybir.dt
bf16 = ml_dtypes.bfloat16

BUFS = dict(msg_lo=6, msg_hi=3, msg_lo2=6, idxp=8, spp=16, blkp=7,
            psh=2, pstr=2, psout=3)

_prog_cache = {}


# ---------------- host-side prep ----------------

def _classify(pos, src, npos):
    """Per-position (must-lo, must-hi, flexible) source counts."""
    ml = np.bincount(pos[src < HI_BASE], minlength=npos)
    mh = np.bincount(pos[src >= LO_LIMIT], minlength=npos)
    fx = np.bincount(pos[(src >= HI_BASE) & (src < LO_LIMIT)], minlength=npos)
    return ml, mh, fx


def _choose_chunks(cands, npos):
    """Unified per-position (L, H) chunk counts covering every variant in
    `cands` (list of (ml, mh, fx) triples), minimizing L+H; plus each
    variant's flex-to-lo counts."""
    tots = [ml + mh + fx for (ml, mh, fx) in cands]
    L = np.zeros(npos, np.int64)
    H = np.zeros(npos, np.int64)
    for p in range(npos):
        lmin = max((int(ml[p]) + P - 1) // P for (ml, _, _) in cands)
        lmax = min(int(ml[p] + fx[p]) // P for (ml, _, fx) in cands)
        lfull = max((int(t[p]) + P - 1) // P for t in tots)
        best = None
        for Lc in range(lmin, max(lmin, lmax, lfull) + 2):
            need = 0
            for (ml, mh, fx), tot in zip(cands, tots):
                rem = int(tot[p]) - min(Lc * P, int(ml[p] + fx[p]))
                need = max(need, rem, int(mh[p]))
            Hc = (need + P - 1) // P
            # <= so ties prefer the larger lo stream (keeps hop-2 hi empty)
            if best is None or Lc + Hc <= best[0] + best[1]:
                best = (Lc, Hc)
        L[p], H[p] = best
    L[(L + H) == 0] = 1   # keep every position's PSUM block defined
    f2l = [np.minimum(L * P, ml + fx) - ml for (ml, _, fx) in cands]
    return L, H, f2l


def _build_stream(pos, dstloc, src, nv, L, H, flex_to_lo, npos):
    """Padded token streams + chunk-major meta for one shard variant.

    Position p's lo tokens occupy lo-stream slots [cumsum, +L[p]*128), hi
    tokens likewise. Device chunk c = chunk_off[p]+j uses lo chunk j for
    j < L[p], else hi chunk j-L[p]. Padding tokens: idx 0 / nv 0 / dst 0.
    """
    lo_chunk_off = np.concatenate([[0], np.cumsum(L)[:-1]])
    hi_chunk_off = np.concatenate([[0], np.cumsum(H)[:-1]])
    chunk_off = np.concatenate([[0], np.cumsum(L + H)[:-1]])
    NCH = int((L + H).sum())
    TLO, THI = int(L.sum()) * P, int(H.sum()) * P

    lo = src < HI_BASE
    flex = (src >= HI_BASE) & (src < LO_LIMIT)
    fidx = np.flatnonzero(flex)
    forder = np.argsort(pos[fidx], kind="stable")
    fpos = pos[fidx[forder]]
    fcnt = np.bincount(fpos, minlength=npos)
    fstart = np.concatenate([[0], np.cumsum(fcnt)[:-1]])
    frank = np.arange(fidx.size) - fstart[fpos]
    lo = lo.copy()
    lo[fidx[forder]] = frank < flex_to_lo[fpos]
    assert (np.bincount(pos[lo], minlength=npos) <= L * P).all()
    assert (np.bincount(pos[~lo], minlength=npos) <= H * P).all()
    order = np.lexsort((~lo, pos))
    dl_s, s_s, nv_s = dstloc[order], src[order], nv[order]
    pos_s, lo_s = pos[order], lo[order]
    gid = pos_s * 2 + (~lo_s).astype(np.int64)
    cnt = np.bincount(gid, minlength=npos * 2)
    gstart = np.concatenate([[0], np.cumsum(cnt)[:-1]])
    rank = np.arange(dl_s.size) - gstart[gid]
    slot = np.where(lo_s, lo_chunk_off[pos_s] * P + rank,
                    hi_chunk_off[pos_s] * P + rank)

    idx_lo = np.zeros(TLO, np.int16)
    nv_lo = np.zeros(TLO, np.float32)
    rm_lo = np.zeros(TLO, np.float32)
    idx_hi = np.zeros(THI, np.int16)
    nv_hi = np.zeros(THI, np.float32)
    rm_hi = np.zeros(THI, np.float32)
    m = lo_s
    idx_lo[slot[m]] = s_s[m].astype(np.int16)
    nv_lo[slot[m]] = nv_s[m]
    rm_lo[slot[m]] = dl_s[m].astype(np.float32)
    m = ~lo_s
    idx_hi[slot[m]] = (s_s[m] - HI_BASE).astype(np.int16)
    nv_hi[slot[m]] = nv_s[m]
    rm_hi[slot[m]] = dl_s[m].astype(np.float32)

    rowm = np.zeros((P, NCH), np.float32)
    nvm = np.zeros((P, NCH), np.float32)
    lo_cols = (np.repeat(chunk_off, L) +
               (np.arange(TLO // P) - np.repeat(lo_chunk_off, L))) if TLO else []
    hi_cols = (np.repeat(chunk_off + L, H) +
               (np.arange(THI // P) - np.repeat(hi_chunk_off, H))) if THI else []
    if TLO:
        rowm[:, lo_cols] = rm_lo.reshape(-1, P).T
        nvm[:, lo_cols] = nv_lo.reshape(-1, P).T
    if THI:
        rowm[:, hi_cols] = rm_hi.reshape(-1, P).T
        nvm[:, hi_cols] = nv_hi.reshape(-1, P).T

    def wrap(a):  # [T] -> [128, T/16]; token i at [i%16, i//16], replicated 8x
        if a.size == 0:
            return np.zeros((P, 0), np.int16)
        return np.ascontiguousarray(np.tile(a.reshape(a.size // 16, 16).T, (8, 1)))

    return wrap(idx_lo), wrap(idx_hi), rowm, nvm


def _pack_positions(tot, blocks, npos):
    """Assign `blocks` to program positions by descending token count.
    Returns (order, inv) where order[p] = absolute block (-1 pad) and
    inv[blk] = position."""
    o = blocks[np.argsort(-tot[blocks], kind="stable")]
    order = np.full(npos, -1, np.int64)
    order[:o.size] = o
    inv = np.full(NB, -1, np.int64)
    inv[o] = np.arange(o.size)
    return order, inv


# ---------------- device program (SPMD over the 8 cores) ----------------

def _build_program(L1, H1, L2):
    NCH1 = int((L1 + H1).sum())
    NCH2 = int(L2.sum())
    TLO1, THI1 = int(L1.sum()) * P, int(H1.sum()) * P
    T2 = NCH2 * P
    nc = bacc.Bacc("TRN2", target_bir_lowering=False, debug=False, num_devices=1)
    x2 = nc.dram_tensor("x2", [NNP, TOKC], dt.bfloat16, kind="ExternalInput")
    w2_d = nc.dram_tensor("w2", [P, 2, P], dt.bfloat16, kind="ExternalInput")
    idx_d = {
        'lo': nc.dram_tensor("idx_lo", [P, TLO1 // 16], dt.int16, kind="ExternalInput"),
        'hi': nc.dram_tensor("idx_hi", [P, THI1 // 16], dt.int16, kind="ExternalInput"),
        'lo2': nc.dram_tensor("idx2", [P, T2 // 16], dt.int16, kind="ExternalInput"),
    }
    rowm1_d = nc.dram_tensor("rowm1", [P, NCH1], dt.float32, kind="ExternalInput")
    nvm1_d = nc.dram_tensor("nvm1", [P, NCH1], dt.float32, kind="ExternalInput")
    rowm2_d = nc.dram_tensor("rowm2", [P, NCH2], dt.float32, kind="ExternalInput")
    nvm2_d = nc.dram_tensor("nvm2", [P, NCH2], dt.float32, kind="ExternalInput")
    z2 = nc.dram_tensor("z2", [HALF_ROWS, TOKC], dt.bfloat16)
    outA = nc.dram_tensor("outA", [HALF_ROWS, TOKC], dt.bfloat16, kind="ExternalOutput")
    outB = nc.dram_tensor("outB", [NNP, TOKC], dt.bfloat16, kind="ExternalOutput")

    with tile.TileContext(nc) as tc:
        with (tc.tile_pool(name="const", bufs=1) as constp,
              tc.tile_pool(name="meta", bufs=1) as metap,
              tc.tile_pool(name="msg_lo", bufs=BUFS["msg_lo"]) as msglop,
              tc.tile_pool(name="msg_hi", bufs=BUFS["msg_hi"]) as msghip,
              tc.tile_pool(name="msg_lo2", bufs=BUFS["msg_lo2"]) as msglo2p,
              tc.tile_pool(name="idxp", bufs=BUFS["idxp"]) as idxp,
              tc.tile_pool(name="spp", bufs=BUFS["spp"]) as spp,
              tc.tile_pool(name="blkp", bufs=BUFS["blkp"]) as blkp,
              tc.tile_pool(name="psh", bufs=BUFS["psh"], space="PSUM") as psum_h,
              tc.tile_pool(name="pstr", bufs=BUFS["pstr"], space="PSUM") as psum_tr,
              tc.tile_pool(name="psout", bufs=BUFS["psout"], space="PSUM") as psum_out):

            iota_i = constp.tile([P, P], dt.int32)
            nc.gpsimd.iota(iota_i[:], pattern=[[1, P]], base=0, channel_multiplier=0)
            iota_f = constp.tile([P, P], dt.bfloat16)
            nc.vector.tensor_copy(iota_f[:], iota_i[:])
            ident = constp.tile([P, P], dt.bfloat16)
            make_identity(nc, ident[:])
            w2_sb = constp.tile([P, 2, P], dt.bfloat16)
            nc.sync.dma_start(out=w2_sb[:], in_=w2_d[:])
            rowm1_sb = metap.tile([P, NCH1], dt.float32)
            nc.sync.dma_start(out=rowm1_sb[:], in_=rowm1_d[:])
            nvm1_sb = metap.tile([P, NCH1], dt.float32)
            nc.sync.dma_start(out=nvm1_sb[:], in_=nvm1_d[:])
            rowm2_sb = metap.tile([P, NCH2], dt.float32)
            nc.sync.dma_start(out=rowm2_sb[:], in_=rowm2_d[:])
            nvm2_sb = metap.tile([P, NCH2], dt.float32)
            nc.sync.dma_start(out=nvm2_sb[:], in_=nvm2_d[:])

            slab_cache = {}

            def get_chunk(stream, src_ap, pool, T, gpos):
                tile_obj, s_cur = slab_cache.get(stream, (None, -1))
                s, j = divmod(gpos, GATHER_SLAB // P)
                if s != s_cur:
                    off = s * GATHER_SLAB
                    g = min(GATHER_SLAB, T - off)
                    it = idxp.tile([P, g // 16], dt.int16, tag="idx")
                    nc.sync.dma_start(
                        out=it[:], in_=idx_d[stream][:, off // 16:(off + g) // 16])
                    mt = pool.tile([P, g // P, TOKC], dt.bfloat16, tag="m" + stream)
                    nc.gpsimd.dma_gather(
                        out_ap=mt[:], in_ap=src_ap,
                        idxs_ap=it[:], num_idxs=g, num_idxs_reg=g,
                        elem_size=TOKC, single_packet=False)
                    slab_cache[stream] = (mt, s)
                    tile_obj = mt
                return tile_obj, j

            def build_sp(rowm_sb, nvm_sb, c):
                sp = spp.tile([P, P], dt.bfloat16, tag="sp")
                nc.vector.tensor_scalar(
                    sp[:], iota_f[:],
                    rowm_sb[:, c:c + 1], nvm_sb[:, c:c + 1],
                    mybir.AluOpType.is_equal, mybir.AluOpType.mult)
                return sp

            # ---- hop 1: h1[half] = (D^-1 A) h0;  outA = h1 @ W[0] and
            # z2 = h1 @ W[1] (W commutes with A: out = h1 W0 + A (h1 W1),
            # so hop 2 needs no W stage at all). ----
            c = 0
            glo = 0
            ghi = 0
            for p in range(NPOS1):
                Lp, Hp = int(L1[p]), int(H1[p])
                CPB = Lp + Hp
                hp = psum_h.tile([P, 2, P], dt.float32, tag="hp")
                for j in range(CPB):
                    if j < Lp:
                        mt, jj = get_chunk('lo', x2[0:LO_LIMIT, :], msglop,
                                           TLO1, glo + j)
                    else:
                        mt, jj = get_chunk('hi', x2[HI_BASE:NNP, :], msghip,
                                           THI1, ghi + (j - Lp))
                    sp = build_sp(rowm1_sb, nvm1_sb, c + j)
                    nc.tensor.matmul(hp[:], sp[:], mt[:, jj, :],
                                     start=(j == 0), stop=(j == CPB - 1))
                c += CPB
                glo += Lp
                ghi += Hp
                h_sb = blkp.tile([P, 2, P], dt.bfloat16, tag="h_sb")
                nc.scalar.copy(h_sb[:], hp[:])
                tr = psum_tr.tile([P, 2, P], dt.bfloat16, tag="tr")
                nc.tensor.transpose(tr[:, 0, :], h_sb[:, 0, :], ident[:])
                nc.tensor.transpose(tr[:, 1, :], h_sb[:, 1, :], ident[:])
                trs = blkp.tile([P, 2, P], dt.bfloat16, tag="trs")
                nc.vector.tensor_copy(trs[:], tr[:])
                for k, dest in ((1, z2), (0, outA)):
                    op = psum_out.tile([P, 2, P], dt.float32, tag="op")
                    nc.tensor.matmul(op[:, 0, :], trs[:, 0, :], w2_sb[:, k, :],
                                     start=True, stop=True)
                    nc.tensor.matmul(op[:, 1, :], trs[:, 1, :], w2_sb[:, k, :],
                                     start=True, stop=True)
                    ob = blkp.tile([P, 2, P], dt.bfloat16, tag="ob")
                    if k == 1:
                        nc.scalar.copy(ob[:], op[:])
                    else:
                        nc.vector.tensor_copy(ob[:], op[:])
                    nc.sync.dma_start(out=dest[p * P:(p + 1) * P, :], in_=ob[:])

            # ---- hop 2: outB = (D^-1 A)|src-half z2 (final partial) ----
            # The barrier orders hop-2's z2 gathers after hop-1's z2 writes
            # (DRAM RAW is not tracked at tile granularity).
            tc.strict_bb_all_engine_barrier()
            c = 0
            for p in range(NB):
                CPB = int(L2[p])
                hp = psum_h.tile([P, 2, P], dt.float32, tag="hp")
                for j in range(CPB):
                    mt, jj = get_chunk('lo2', z2[0:HALF_ROWS, :], msglo2p, T2,
                                       c + j)
                    sp = build_sp(rowm2_sb, nvm2_sb, c + j)
                    nc.tensor.matmul(hp[:], sp[:], mt[:, jj, :],
                                     start=(j == 0), stop=(j == CPB - 1))
                c += CPB
                ob = blkp.tile([P, 2, P], dt.bfloat16, tag="ob")
                nc.scalar.copy(ob[:], hp[:])
                nc.sync.dma_start(out=outB[p * P:(p + 1) * P, :], in_=ob[:])

    nc.compile()
    return nc


# ---------------- entry point ----------------

def kernel(x, edge_index, edge_vals, W_f, W_b, bias):
    x = np.asarray(x, dtype=np.float32)
    edge_index = np.asarray(edge_index)
    edge_vals = np.asarray(edge_vals, dtype=np.float32)
    W_f = np.asarray(W_f, dtype=np.float32)
    W_b = np.asarray(W_b, dtype=np.float32)
    bias = np.asarray(bias, dtype=np.float32)

    rows = edge_index[0].astype(np.int64)
    cols = edge_index[1].astype(np.int64)
    deg = np.zeros(N_NODES, np.float32)
    np.add.at(deg, rows, edge_vals)
    deg += np.float32(1e-8)
    nv = (edge_vals / deg[rows]).astype(np.float32)

    halves = [np.arange(0, NPOS1), np.arange(NPOS1, NB)]
    v1 = []   # hop-1 variants: (pos, dstloc, src, nv, order)
    v2 = []   # hop-2 variants: (pos, dstloc, srcloc, nv, order, inv1)
    for d in range(2):
        dst, src = (rows, cols) if d == 0 else (cols, rows)
        dblk = dst >> 7
        dloc = dst & (P - 1)
        sblk = src >> 7
        tot1 = np.bincount(dblk, minlength=NB)
        tot2 = np.bincount(dblk, weights=(sblk >= NPOS1).astype(np.float64),
                           minlength=NB)
        for h in range(2):
            sel = (dblk >= NPOS1) == (h == 1)
            order1, inv1 = _pack_positions(tot1, halves[h], NPOS1)
            v1.append((inv1[dblk[sel]], dloc[sel], src[sel], nv[sel], order1))
            sel2 = (sblk >= NPOS1) == (h == 1)
            t2 = tot2 if h == 1 else (tot1 - tot2)
            order2, inv2 = _pack_positions(t2, np.arange(NB), NB)
            srcloc = inv1[sblk[sel2]] * P + (src[sel2] & (P - 1))
            v2.append((inv2[dblk[sel2]], dloc[sel2], srcloc, nv[sel2],
                       order2, inv1))

    c1 = [_classify(pos, src, NPOS1) for (pos, _, src, _, _) in v1]
    L1, H1, f2l1 = _choose_chunks(c1, NPOS1)
    c2 = [_classify(pos, src, NB) for (pos, _, src, _, _, _) in v2]
    L2, H2, f2l2 = _choose_chunks(c2, NB)
    assert H2.sum() == 0, "hop-2 sources must fit the lo stream"

    s1 = [_build_stream(pos, dl, src, nvv, L1, H1, f2l1[i], NPOS1)
          for i, (pos, dl, src, nvv, _) in enumerate(v1)]
    s2 = [_build_stream(pos, dl, src, nvv, L2, H2, f2l2[i], NB)
          for i, (pos, dl, src, nvv, _, _) in enumerate(v2)]

    key = (L1.tobytes(), H1.tobytes(), L2.tobytes())
    if key not in _prog_cache:
        _prog_cache.clear()
        _prog_cache[key] = _build_program(L1, H1, L2)
    nc = _prog_cache[key]

    in_maps = []
    for core in range(8):
        d, g, h = core >> 2, (core >> 1) & 1, core & 1
        vi = d * 2 + h
        Wd = W_f if d == 0 else W_b
        x2 = np.zeros((NNP, TOKC), bf16)
        x2[:N_NODES] = x[4 * g:4 * g + 4].transpose(1, 0, 2).reshape(
            N_NODES, TOKC).astype(bf16)
        w2 = np.zeros((P, 2, P), bf16)
        for k in range(2):
            for a in range(2):
                w2[C * a:C * a + C, k, C * a:C * a + C] = Wd[k].astype(bf16)
        in_maps.append({
            "x2": x2, "w2": w2,
            "idx_lo": s1[vi][0], "idx_hi": s1[vi][1],
            "rowm1": s1[vi][2], "nvm1": s1[vi][3],
            "idx2": s2[vi][0],
            "rowm2": s2[vi][2], "nvm2": s2[vi][3],
        })

    results = run_bass_kernel_spmd(nc, in_maps, list(range(8))).results

    out = np.empty((B, N_NODES, C), np.float32)
    for g in range(2):
        acc = np.zeros((NNP, TOKC), np.float32)
        for d in range(2):
            for h in range(2):
                vi = d * 2 + h
                r = results[(d << 2) | (g << 1) | h]
                order2 = v2[vi][4]
                inv2 = np.argsort(order2)
                acc += np.asarray(r["outB"]).astype(np.float32).reshape(
                    NB, P, TOKC)[inv2].reshape(NNP, TOKC)
                order1 = v1[vi][4]
                nreal = halves[h].size
                oa = np.asarray(r["outA"]).astype(np.float32).reshape(
                    NPOS1, P, TOKC)[:nreal]
                accb = acc.reshape(NB, P, TOKC)
                accb[order1[:nreal]] += oa
        for bl in range(4):
            out[4 * g + bl] = acc[:N_NODES, C * bl:C * bl + C]
    out += bias.reshape(1, 1, C)
    return out


# revision 23
# speedup vs baseline: 1.1409x; 1.0269x over previous
"""DiffusionGraphConv on 8 Trainium2 NeuronCores (Bass/Tile).

out = sum_k (D^-1 A)^k x W_f[k] + ((D^-1 A)^T)^k x W_b[k] + bias, K=2,
N=50000 nodes, E=800000 edges, B=8, C_in=C_out=64, f32.

Sharding: 8 cores = 2 diffusion directions x 2 batch-groups (4 batches
packed per 512B bf16 gather token) x 2 node-halves. No cross-core
traffic: hop 1 processes edges whose DESTINATION block falls in the
core's half (gathering from the replicated x), producing that half of
h1 = (D^-1 A) h0; hop 2 processes edges whose SOURCE falls in the same
half (gathering only from the core's own h1) and scatter-adds into all
destination blocks. The four partial outputs per batch-group (2 dirs x
2 halves) are summed on the host together with the bias.

Per hop on device: messages h[src[e]] are fetched with nc.gpsimd.dma_gather
(512B bf16 tokens); the scatter-add is a TensorE matmul per 128-edge chunk
with a one-hot matrix S[t,n] = (n == dst_local[t]) * nv[t] built by one DVE
tensor_scalar(is_equal, mult) op in bf16 (4x DVE mode); chunks accumulate
per 128-row node block in PSUM. Hop 2 accumulates the transposed block
(lhsT=chunk) directly so no PE transpose is needed before the W matmul.

Blocks are assigned to program positions per-core by descending chunk
count (bin-packing) so one SPMD program's per-position chunk counts,
taken as the max over the 4 edge-shard variants, waste little padding.
"""
import numpy as np
import ml_dtypes

import concourse.bacc as bacc
import concourse.tile as tile
import concourse.mybir as mybir
from concourse.bass_utils import run_bass_kernel_spmd
from concourse.masks import make_identity

P = 128
N_NODES = 50000
N_EDGES = 800000
B, C = 8, 64
NNP = 50048          # nodes padded to a multiple of 128
NB = NNP // P        # 391 destination blocks
NPOS1 = 196          # hop-1 program positions (half0: 196 blocks, half1: 195)
HALF_ROWS = NPOS1 * P   # 25088 rows of h1 per core
LO_LIMIT = 32768     # src < LO_LIMIT -> lo gather stream (int16 idx range)
HI_BASE = NNP - 32768   # hi stream gathers rows [HI_BASE:], idx = src - HI_BASE
GATHER_SLAB = 2048   # tokens per dma_gather instruction
TOKC = 4 * C         # 256 bf16 values per token (4 batches x 64 ch) = 512B
dt = mybir.dt
bf16 = ml_dtypes.bfloat16

BUFS = dict(msg_lo=6, msg_hi=3, msg_lo2=6, idxp=8, spp=16, blkp=7,
            psh=2, pstr=2, psout=3)

_prog_cache = {}


# ---------------- host-side prep ----------------

def _classify(pos, src, npos):
    """Per-position (must-lo, must-hi, flexible) source counts."""
    ml = np.bincount(pos[src < HI_BASE], minlength=npos)
    mh = np.bincount(pos[src >= LO_LIMIT], minlength=npos)
    fx = np.bincount(pos[(src >= HI_BASE) & (src < LO_LIMIT)], minlength=npos)
    return ml, mh, fx


def _choose_chunks(cands, npos):
    """Unified per-position (L, H) chunk counts covering every variant in
    `cands` (list of (ml, mh, fx) triples), minimizing L+H; plus each
    variant's flex-to-lo counts."""
    tots = [ml + mh + fx for (ml, mh, fx) in cands]
    L = np.zeros(npos, np.int64)
    H = np.zeros(npos, np.int64)
    for p in range(npos):
        lmin = max((int(ml[p]) + P - 1) // P for (ml, _, _) in cands)
        lmax = min(int(ml[p] + fx[p]) // P for (ml, _, fx) in cands)
        lfull = max((int(t[p]) + P - 1) // P for t in tots)
        best = None
        for Lc in range(lmin, max(lmin, lmax, lfull) + 2):
            need = 0
            for (ml, mh, fx), tot in zip(cands, tots):
                rem = int(tot[p]) - min(Lc * P, int(ml[p] + fx[p]))
                need = max(need, rem, int(mh[p]))
            Hc = (need + P - 1) // P
            # <= so ties prefer the larger lo stream (keeps hop-2 hi empty)
            if best is None or Lc + Hc <= best[0] + best[1]:
                best = (Lc, Hc)
        L[p], H[p] = best
    L[(L + H) == 0] = 1   # keep every position's PSUM block defined
    f2l = [np.minimum(L * P, ml + fx) - ml for (ml, _, fx) in cands]
    return L, H, f2l


def _build_stream(pos, dstloc, src, nv, L, H, flex_to_lo, npos):
    """Padded token streams + chunk-major meta for one shard variant.

    Position p's lo tokens occupy lo-stream slots [cumsum, +L[p]*128), hi
    tokens likewise. Device chunk c = chunk_off[p]+j uses lo chunk j for
    j < L[p], else hi chunk j-L[p]. Padding tokens: idx 0 / nv 0 / dst 0.
    """
    lo_chunk_off = np.concatenate([[0], np.cumsum(L)[:-1]])
    hi_chunk_off = np.concatenate([[0], np.cumsum(H)[:-1]])
    chunk_off = np.concatenate([[0], np.cumsum(L + H)[:-1]])
    NCH = int((L + H).sum())
    TLO, THI = int(L.sum()) * P, int(H.sum()) * P

    lo = src < HI_BASE
    flex = (src >= HI_BASE) & (src < LO_LIMIT)
    fidx = np.flatnonzero(flex)
    forder = np.argsort(pos[fidx], kind="stable")
    fpos = pos[fidx[forder]]
    fcnt = np.bincount(fpos, minlength=npos)
    fstart = np.concatenate([[0], np.cumsum(fcnt)[:-1]])
    frank = np.arange(fidx.size) - fstart[fpos]
    lo = lo.copy()
    lo[fidx[forder]] = frank < flex_to_lo[fpos]
    assert (np.bincount(pos[lo], minlength=npos) <= L * P).all()
    assert (np.bincount(pos[~lo], minlength=npos) <= H * P).all()
    order = np.lexsort((~lo, pos))
    dl_s, s_s, nv_s = dstloc[order], src[order], nv[order]
    pos_s, lo_s = pos[order], lo[order]
    gid = pos_s * 2 + (~lo_s).astype(np.int64)
    cnt = np.bincount(gid, minlength=npos * 2)
    gstart = np.concatenate([[0], np.cumsum(cnt)[:-1]])
    rank = np.arange(dl_s.size) - gstart[gid]
    slot = np.where(lo_s, lo_chunk_off[pos_s] * P + rank,
                    hi_chunk_off[pos_s] * P + rank)

    idx_lo = np.zeros(TLO, np.int16)
    nv_lo = np.zeros(TLO, np.float32)
    rm_lo = np.zeros(TLO, np.float32)
    idx_hi = np.zeros(THI, np.int16)
    nv_hi = np.zeros(THI, np.float32)
    rm_hi = np.zeros(THI, np.float32)
    m = lo_s
    idx_lo[slot[m]] = s_s[m].astype(np.int16)
    nv_lo[slot[m]] = nv_s[m]
    rm_lo[slot[m]] = dl_s[m].astype(np.float32)
    m = ~lo_s
    idx_hi[slot[m]] = (s_s[m] - HI_BASE).astype(np.int16)
    nv_hi[slot[m]] = nv_s[m]
    rm_hi[slot[m]] = dl_s[m].astype(np.float32)

    rowm = np.zeros((P, NCH), np.float32)
    nvm = np.zeros((P, NCH), np.float32)
    lo_cols = (np.repeat(chunk_off, L) +
               (np.arange(TLO // P) - np.repeat(lo_chunk_off, L))) if TLO else []
    hi_cols = (np.repeat(chunk_off + L, H) +
               (np.arange(THI // P) - np.repeat(hi_chunk_off, H))) if THI else []
    if TLO:
        rowm[:, lo_cols] = rm_lo.reshape(-1, P).T
        nvm[:, lo_cols] = nv_lo.reshape(-1, P).T
    if THI:
        rowm[:, hi_cols] = rm_hi.reshape(-1, P).T
        nvm[:, hi_cols] = nv_hi.reshape(-1, P).T

    def wrap(a):  # [T] -> [128, T/16]; token i at [i%16, i//16], replicated 8x
        if a.size == 0:
            return np.zeros((P, 0), np.int16)
        return np.ascontiguousarray(np.tile(a.reshape(a.size // 16, 16).T, (8, 1)))

    return wrap(idx_lo), wrap(idx_hi), rowm, nvm


def _build_stream2(pos, dstloc, src, nv, mt2):
    """Continuous (boundary-sharing) token stream for hop 2.

    Position p's tokens occupy slots [off[p], off[p]+mt2[p]); chunk
    boundaries fall anywhere, so a chunk shared by two positions appears
    in both positions' meta columns with complementary nv=0 masking.
    """
    npos = mt2.size
    off = np.concatenate([[0], np.cumsum(mt2)[:-1]])
    T2 = int(((mt2.sum() + P - 1) // P) * P)
    cs = off >> 7
    ce = (off + mt2 - 1) >> 7
    ce[-1] = (T2 - 1) >> 7
    cco = np.concatenate([[0], np.cumsum(ce - cs + 1)[:-1]])
    NCH = int((ce - cs + 1).sum())

    order = np.argsort(pos, kind="stable")
    pos_s, dl_s, s_s, nv_s = pos[order], dstloc[order], src[order], nv[order]
    cnt = np.bincount(pos_s, minlength=npos)
    assert (cnt <= mt2).all()
    gstart = np.concatenate([[0], np.cumsum(cnt)[:-1]])
    rank = np.arange(pos_s.size) - gstart[pos_s]
    slot = off[pos_s] + rank

    idx = np.zeros(T2, np.int16)
    idx[slot] = s_s.astype(np.int16)
    rowm = np.zeros((P, NCH), np.float32)
    nvm = np.zeros((P, NCH), np.float32)
    col = cco[pos_s] + (slot >> 7) - cs[pos_s]
    rowm[slot & (P - 1), col] = dl_s.astype(np.float32)
    nvm[slot & (P - 1), col] = nv_s

    def wrap(a):
        return np.ascontiguousarray(np.tile(a.reshape(a.size // 16, 16).T, (8, 1)))

    return wrap(idx), rowm, nvm


def _pack_positions(tot, blocks, npos):
    """Assign `blocks` to program positions by descending token count.
    Returns (order, inv) where order[p] = absolute block (-1 pad) and
    inv[blk] = position."""
    o = blocks[np.argsort(-tot[blocks], kind="stable")]
    order = np.full(npos, -1, np.int64)
    order[:o.size] = o
    inv = np.full(NB, -1, np.int64)
    inv[o] = np.arange(o.size)
    return order, inv


# ---------------- device program (SPMD over the 8 cores) ----------------

def _build_program(L1, H1, mt2):
    NCH1 = int((L1 + H1).sum())
    TLO1, THI1 = int(L1.sum()) * P, int(H1.sum()) * P
    off2 = np.concatenate([[0], np.cumsum(mt2)[:-1]])
    T2 = int(((mt2.sum() + P - 1) // P) * P)
    cs2 = off2 >> 7
    ce2 = (off2 + mt2 - 1) >> 7
    ce2[-1] = (T2 - 1) >> 7
    cco2 = np.concatenate([[0], np.cumsum(ce2 - cs2 + 1)[:-1]])
    NCH2 = int((ce2 - cs2 + 1).sum())
    nc = bacc.Bacc("TRN2", target_bir_lowering=False, debug=False, num_devices=1)
    x2 = nc.dram_tensor("x2", [NNP, TOKC], dt.bfloat16, kind="ExternalInput")
    w2_d = nc.dram_tensor("w2", [P, 2, P], dt.bfloat16, kind="ExternalInput")
    idx_d = {
        'lo': nc.dram_tensor("idx_lo", [P, TLO1 // 16], dt.int16, kind="ExternalInput"),
        'hi': nc.dram_tensor("idx_hi", [P, THI1 // 16], dt.int16, kind="ExternalInput"),
        'lo2': nc.dram_tensor("idx2", [P, T2 // 16], dt.int16, kind="ExternalInput"),
    }
    rowm1_d = nc.dram_tensor("rowm1", [P, NCH1], dt.float32, kind="ExternalInput")
    nvm1_d = nc.dram_tensor("nvm1", [P, NCH1], dt.float32, kind="ExternalInput")
    rowm2_d = nc.dram_tensor("rowm2", [P, NCH2], dt.float32, kind="ExternalInput")
    nvm2_d = nc.dram_tensor("nvm2", [P, NCH2], dt.float32, kind="ExternalInput")
    z2 = nc.dram_tensor("z2", [HALF_ROWS, TOKC], dt.bfloat16)
    outA = nc.dram_tensor("outA", [HALF_ROWS, TOKC], dt.bfloat16, kind="ExternalOutput")
    outB = nc.dram_tensor("outB", [NNP, TOKC], dt.bfloat16, kind="ExternalOutput")

    with tile.TileContext(nc) as tc:
        with (tc.tile_pool(name="const", bufs=1) as constp,
              tc.tile_pool(name="meta", bufs=1) as metap,
              tc.tile_pool(name="msg_lo", bufs=BUFS["msg_lo"]) as msglop,
              tc.tile_pool(name="msg_hi", bufs=BUFS["msg_hi"]) as msghip,
              tc.tile_pool(name="msg_lo2", bufs=BUFS["msg_lo2"]) as msglo2p,
              tc.tile_pool(name="idxp", bufs=BUFS["idxp"]) as idxp,
              tc.tile_pool(name="spp", bufs=BUFS["spp"]) as spp,
              tc.tile_pool(name="blkp", bufs=BUFS["blkp"]) as blkp,
              tc.tile_pool(name="psh", bufs=BUFS["psh"], space="PSUM") as psum_h,
              tc.tile_pool(name="pstr", bufs=BUFS["pstr"], space="PSUM") as psum_tr,
              tc.tile_pool(name="psout", bufs=BUFS["psout"], space="PSUM") as psum_out):

            iota_i = constp.tile([P, P], dt.int32)
            nc.gpsimd.iota(iota_i[:], pattern=[[1, P]], base=0, channel_multiplier=0)
            iota_f = constp.tile([P, P], dt.bfloat16)
            nc.vector.tensor_copy(iota_f[:], iota_i[:])
            ident = constp.tile([P, P], dt.bfloat16)
            make_identity(nc, ident[:])
            w2_sb = constp.tile([P, 2, P], dt.bfloat16)
            nc.sync.dma_start(out=w2_sb[:], in_=w2_d[:])
            rowm1_sb = metap.tile([P, NCH1], dt.float32)
            nc.sync.dma_start(out=rowm1_sb[:], in_=rowm1_d[:])
            nvm1_sb = metap.tile([P, NCH1], dt.float32)
            nc.sync.dma_start(out=nvm1_sb[:], in_=nvm1_d[:])
            rowm2_sb = metap.tile([P, NCH2], dt.float32)
            nc.sync.dma_start(out=rowm2_sb[:], in_=rowm2_d[:])
            nvm2_sb = metap.tile([P, NCH2], dt.float32)
            nc.sync.dma_start(out=nvm2_sb[:], in_=nvm2_d[:])

            slab_cache = {}

            def get_chunk(stream, src_ap, pool, T, gpos):
                tile_obj, s_cur = slab_cache.get(stream, (None, -1))
                s, j = divmod(gpos, GATHER_SLAB // P)
                if s != s_cur:
                    off = s * GATHER_SLAB
                    g = min(GATHER_SLAB, T - off)
                    it = idxp.tile([P, g // 16], dt.int16, tag="idx")
                    nc.sync.dma_start(
                        out=it[:], in_=idx_d[stream][:, off // 16:(off + g) // 16])
                    mt = pool.tile([P, g // P, TOKC], dt.bfloat16, tag="m" + stream)
                    nc.gpsimd.dma_gather(
                        out_ap=mt[:], in_ap=src_ap,
                        idxs_ap=it[:], num_idxs=g, num_idxs_reg=g,
                        elem_size=TOKC, single_packet=False)
                    slab_cache[stream] = (mt, s)
                    tile_obj = mt
                return tile_obj, j

            def build_sp(rowm_sb, nvm_sb, c):
                sp = spp.tile([P, P], dt.bfloat16, tag="sp")
                nc.vector.tensor_scalar(
                    sp[:], iota_f[:],
                    rowm_sb[:, c:c + 1], nvm_sb[:, c:c + 1],
                    mybir.AluOpType.is_equal, mybir.AluOpType.mult)
                return sp

            # ---- hop 1: h1[half] = (D^-1 A) h0;  outA = h1 @ W[0] and
            # z2 = h1 @ W[1] (W commutes with A: out = h1 W0 + A (h1 W1),
            # so hop 2 needs no W stage at all). ----
            c = 0
            glo = 0
            ghi = 0
            for p in range(NPOS1):
                Lp, Hp = int(L1[p]), int(H1[p])
                CPB = Lp + Hp
                hp = psum_h.tile([P, 2, P], dt.float32, tag="hp")
                for j in range(CPB):
                    if j < Lp:
                        mt, jj = get_chunk('lo', x2[0:LO_LIMIT, :], msglop,
                                           TLO1, glo + j)
                    else:
                        mt, jj = get_chunk('hi', x2[HI_BASE:NNP, :], msghip,
                                           THI1, ghi + (j - Lp))
                    sp = build_sp(rowm1_sb, nvm1_sb, c + j)
                    nc.tensor.matmul(hp[:], sp[:], mt[:, jj, :],
                                     start=(j == 0), stop=(j == CPB - 1))
                c += CPB
                glo += Lp
                ghi += Hp
                h_sb = blkp.tile([P, 2, P], dt.bfloat16, tag="h_sb")
                nc.scalar.copy(h_sb[:], hp[:])
                tr = psum_tr.tile([P, 2, P], dt.bfloat16, tag="tr")
                nc.tensor.transpose(tr[:, 0, :], h_sb[:, 0, :], ident[:])
                nc.tensor.transpose(tr[:, 1, :], h_sb[:, 1, :], ident[:])
                trs = blkp.tile([P, 2, P], dt.bfloat16, tag="trs")
                nc.vector.tensor_copy(trs[:], tr[:])
                for k, dest in ((1, z2), (0, outA)):
                    op = psum_out.tile([P, 2, P], dt.float32, tag="op")
                    nc.tensor.matmul(op[:, 0, :], trs[:, 0, :], w2_sb[:, k, :],
                                     start=True, stop=True)
                    nc.tensor.matmul(op[:, 1, :], trs[:, 1, :], w2_sb[:, k, :],
                                     start=True, stop=True)
                    ob = blkp.tile([P, 2, P], dt.bfloat16, tag="ob")
                    if k == 1:
                        nc.scalar.copy(ob[:], op[:])
                    else:
                        nc.vector.tensor_copy(ob[:], op[:])
                    nc.sync.dma_start(out=dest[p * P:(p + 1) * P, :], in_=ob[:])

            # ---- hop 2: outB = (D^-1 A)|src-half z2 (final partial) ----
            # Boundary-sharing stream: chunk ranges [cs2[p], ce2[p]] overlap
            # between adjacent positions; each position has its own nv=0
            # masked meta column for a shared chunk.
            # The barrier orders hop-2's z2 gathers after hop-1's z2 writes
            # (DRAM RAW is not tracked at tile granularity).
            tc.strict_bb_all_engine_barrier()
            for p in range(NB):
                hp = psum_h.tile([P, 2, P], dt.float32, tag="hp")
                for j in range(int(cs2[p]), int(ce2[p]) + 1):
                    mt, jj = get_chunk('lo2', z2[0:HALF_ROWS, :], msglo2p, T2, j)
                    sp = build_sp(rowm2_sb, nvm2_sb,
                                  int(cco2[p]) + j - int(cs2[p]))
                    nc.tensor.matmul(hp[:], sp[:], mt[:, jj, :],
                                     start=(j == int(cs2[p])),
                                     stop=(j == int(ce2[p])))
                ob = blkp.tile([P, 2, P], dt.bfloat16, tag="ob")
                nc.scalar.copy(ob[:], hp[:])
                nc.sync.dma_start(out=outB[p * P:(p + 1) * P, :], in_=ob[:])

    nc.compile()
    return nc


# ---------------- entry point ----------------

def kernel(x, edge_index, edge_vals, W_f, W_b, bias):
    x = np.asarray(x, dtype=np.float32)
    edge_index = np.asarray(edge_index)
    edge_vals = np.asarray(edge_vals, dtype=np.float32)
    W_f = np.asarray(W_f, dtype=np.float32)
    W_b = np.asarray(W_b, dtype=np.float32)
    bias = np.asarray(bias, dtype=np.float32)

    rows = edge_index[0].astype(np.int64)
    cols = edge_index[1].astype(np.int64)
    deg = np.zeros(N_NODES, np.float32)
    np.add.at(deg, rows, edge_vals)
    deg += np.float32(1e-8)
    nv = (edge_vals / deg[rows]).astype(np.float32)

    halves = [np.arange(0, NPOS1), np.arange(NPOS1, NB)]
    v1 = []   # hop-1 variants: (pos, dstloc, src, nv, order)
    v2 = []   # hop-2 variants: (pos, dstloc, srcloc, nv, order, inv1)
    for d in range(2):
        dst, src = (rows, cols) if d == 0 else (cols, rows)
        dblk = dst >> 7
        dloc = dst & (P - 1)
        sblk = src >> 7
        tot1 = np.bincount(dblk, minlength=NB)
        tot2 = np.bincount(dblk, weights=(sblk >= NPOS1).astype(np.float64),
                           minlength=NB)
        for h in range(2):
            sel = (dblk >= NPOS1) == (h == 1)
            order1, inv1 = _pack_positions(tot1, halves[h], NPOS1)
            v1.append((inv1[dblk[sel]], dloc[sel], src[sel], nv[sel], order1))
            sel2 = (sblk >= NPOS1) == (h == 1)
            t2 = tot2 if h == 1 else (tot1 - tot2)
            order2, inv2 = _pack_positions(t2, np.arange(NB), NB)
            srcloc = inv1[sblk[sel2]] * P + (src[sel2] & (P - 1))
            v2.append((inv2[dblk[sel2]], dloc[sel2], srcloc, nv[sel2],
                       order2, inv1))

    c1 = [_classify(pos, src, NPOS1) for (pos, _, src, _, _) in v1]
    L1, H1, f2l1 = _choose_chunks(c1, NPOS1)
    mt2 = np.maximum.reduce([np.bincount(pos, minlength=NB)
                             for (pos, _, src, _, _, _) in v2])
    mt2 = np.maximum(mt2, 1)

    s1 = [_build_stream(pos, dl, src, nvv, L1, H1, f2l1[i], NPOS1)
          for i, (pos, dl, src, nvv, _) in enumerate(v1)]
    s2 = [_build_stream2(pos, dl, src, nvv, mt2)
          for (pos, dl, src, nvv, _, _) in v2]

    key = (L1.tobytes(), H1.tobytes(), mt2.tobytes())
    if key not in _prog_cache:
        _prog_cache.clear()
        _prog_cache[key] = _build_program(L1, H1, mt2)
    nc = _prog_cache[key]

    in_maps = []
    for core in range(8):
        d, g, h = core >> 2, (core >> 1) & 1, core & 1
        vi = d * 2 + h
        Wd = W_f if d == 0 else W_b
        x2 = np.zeros((NNP, TOKC), bf16)
        x2[:N_NODES] = x[4 * g:4 * g + 4].transpose(1, 0, 2).reshape(
            N_NODES, TOKC).astype(bf16)
        w2 = np.zeros((P, 2, P), bf16)
        for k in range(2):
            for a in range(2):
                w2[C * a:C * a + C, k, C * a:C * a + C] = Wd[k].astype(bf16)
        in_maps.append({
            "x2": x2, "w2": w2,
            "idx_lo": s1[vi][0], "idx_hi": s1[vi][1],
            "rowm1": s1[vi][2], "nvm1": s1[vi][3],
            "idx2": s2[vi][0],
            "rowm2": s2[vi][1], "nvm2": s2[vi][2],
        })

    results = run_bass_kernel_spmd(nc, in_maps, list(range(8))).results

    out = np.empty((B, N_NODES, C), np.float32)
    for g in range(2):
        acc = np.zeros((NNP, TOKC), np.float32)
        for d in range(2):
            for h in range(2):
                vi = d * 2 + h
                r = results[(d << 2) | (g << 1) | h]
                order2 = v2[vi][4]
                inv2 = np.argsort(order2)
                acc += np.asarray(r["outB"]).astype(np.float32).reshape(
                    NB, P, TOKC)[inv2].reshape(NNP, TOKC)
                order1 = v1[vi][4]
                nreal = halves[h].size
                oa = np.asarray(r["outA"]).astype(np.float32).reshape(
                    NPOS1, P, TOKC)[:nreal]
                accb = acc.reshape(NB, P, TOKC)
                accb[order1[:nreal]] += oa
        for bl in range(4):
            out[4 * g + bl] = acc[:N_NODES, C * bl:C * bl + C]
    out += bias.reshape(1, 1, C)
    return out


# revision 28
# speedup vs baseline: 1.1560x; 1.0133x over previous
"""DiffusionGraphConv on 8 Trainium2 NeuronCores (Bass/Tile).

out = sum_k (D^-1 A)^k x W_f[k] + ((D^-1 A)^T)^k x W_b[k] + bias, K=2,
N=50000 nodes, E=800000 edges, B=8, C_in=C_out=64, f32.

Sharding: 8 cores = 2 diffusion directions x 2 batch-groups (4 batches
packed per 512B bf16 gather token) x 2 node-halves. No cross-core
traffic: hop 1 processes edges whose DESTINATION block falls in the
core's half (gathering from the replicated x), producing that half of
h1 = (D^-1 A) h0; hop 2 processes edges whose SOURCE falls in the same
half (gathering only from the core's own h1) and scatter-adds into all
destination blocks. The four partial outputs per batch-group (2 dirs x
2 halves) are summed on the host together with the bias.

Per hop on device: messages h[src[e]] are fetched with nc.gpsimd.dma_gather
(512B bf16 tokens); the scatter-add is a TensorE matmul per 128-edge chunk
with a one-hot matrix S[t,n] = (n == dst_local[t]) * nv[t] built by one DVE
tensor_scalar(is_equal, mult) op in bf16 (4x DVE mode); chunks accumulate
per 128-row node block in PSUM. Hop 2 accumulates the transposed block
(lhsT=chunk) directly so no PE transpose is needed before the W matmul.

Blocks are assigned to program positions per-core by descending chunk
count (bin-packing) so one SPMD program's per-position chunk counts,
taken as the max over the 4 edge-shard variants, waste little padding.
"""
import numpy as np
import ml_dtypes

import concourse.bacc as bacc
import concourse.tile as tile
import concourse.mybir as mybir
from concourse.bass_utils import run_bass_kernel_spmd
from concourse.masks import make_identity

P = 128
N_NODES = 50000
N_EDGES = 800000
B, C = 8, 64
NNP = 50048          # nodes padded to a multiple of 128
NB = NNP // P        # 391 destination blocks
NPOS1 = 196          # hop-1 program positions (half0: 196 blocks, half1: 195)
HALF_ROWS = NPOS1 * P   # 25088 rows of h1 per core
LO_LIMIT = 32768     # src < LO_LIMIT -> lo gather stream (int16 idx range)
HI_BASE = NNP - 32768   # hi stream gathers rows [HI_BASE:], idx = src - HI_BASE
GATHER_SLAB = 2048   # tokens per dma_gather instruction
TOKC = 4 * C         # 256 bf16 values per token (4 batches x 64 ch) = 512B
dt = mybir.dt
bf16 = ml_dtypes.bfloat16

BUFS = dict(msg_lo=6, msg_hi=3, msg_lo2=6, idxp=8, spp=16, blkp=7,
            psh=2, pstr=2, psout=3)

_prog_cache = {}


# ---------------- host-side prep ----------------

def _classify(pos, src, npos):
    """Per-position (must-lo, must-hi, flexible) source counts."""
    ml = np.bincount(pos[src < HI_BASE], minlength=npos)
    mh = np.bincount(pos[src >= LO_LIMIT], minlength=npos)
    fx = np.bincount(pos[(src >= HI_BASE) & (src < LO_LIMIT)], minlength=npos)
    return ml, mh, fx


def _choose_split(cands, npos):
    """Per-position unified lo/hi token counts (t_lo, t_hi) covering every
    variant in `cands` ((ml, mh, fx) triples), minimizing t_lo + t_hi via a
    scan over flex thresholds; plus each variant's flex-to-lo counts."""
    nv_ = len(cands)
    t_lo = np.zeros(npos, np.int64)
    t_hi = np.zeros(npos, np.int64)
    f2l = [np.zeros(npos, np.int64) for _ in range(nv_)]
    for p in range(npos):
        mls = [int(ml[p]) for (ml, _, _) in cands]
        fxs = [int(fx[p]) for (_, _, fx) in cands]
        tots = [int((ml + mh + fx)[p]) for (ml, mh, fx) in cands]
        best = None
        for T in sorted({m for m in mls} | {m + f for m, f in zip(mls, fxs)}):
            lo = [min(max(T, m), m + f) for m, f in zip(mls, fxs)]
            hi = [t - lv for t, lv in zip(tots, lo)]
            cost = max(lo) + max(hi)
            if best is None or cost < best[0]:
                best = (cost, max(lo), max(hi), lo)
        t_lo[p] = max(best[1], 1)
        t_hi[p] = best[2]
        for v in range(nv_):
            f2l[v][p] = best[3][v] - mls[v]
    return t_lo, t_hi, f2l


def _geom(t):
    """Continuous-stream geometry for per-position token counts `t`:
    (offsets, padded length, first chunk, last chunk, meta columns)."""
    off = np.concatenate([[0], np.cumsum(t)[:-1]])
    T = int(((int(t.sum()) + P - 1) // P) * P)
    cs = off >> 7
    ce = (off + t - 1) >> 7
    ncols = np.where(t > 0, ce - cs + 1, 0)
    return off, T, cs, ce, ncols


def _wrap(a):  # [T] -> [128, T/16]; token i at [i%16, i//16], replicated 8x
    if a.size == 0:
        return np.zeros((P, 0), np.int16)
    return np.ascontiguousarray(np.tile(a.reshape(a.size // 16, 16).T, (8, 1)))


def _build_stream1(pos, dstloc, src, nv, t_lo, t_hi, flex_to_lo):
    """Boundary-sharing lo/hi token streams + chunk-major meta for one
    hop-1 shard variant. Position p's lo tokens occupy lo-stream slots
    [offL[p], offL[p]+t_lo[p]), hi tokens likewise; chunks shared between
    adjacent positions appear in both positions' meta columns with
    complementary nv=0 masking. Padding tokens: idx 0 / nv 0 / dst 0."""
    npos = t_lo.size
    offL, TL, csL, ceL, ncL = _geom(t_lo)
    offH, TH, csH, ceH, ncH = _geom(t_hi)
    cco = np.concatenate([[0], np.cumsum(ncL + ncH)[:-1]])
    NCH = int((ncL + ncH).sum())

    lo = src < HI_BASE
    flex = (src >= HI_BASE) & (src < LO_LIMIT)
    fidx = np.flatnonzero(flex)
    forder = np.argsort(pos[fidx], kind="stable")
    fpos = pos[fidx[forder]]
    fcnt = np.bincount(fpos, minlength=npos)
    fstart = np.concatenate([[0], np.cumsum(fcnt)[:-1]])
    frank = np.arange(fidx.size) - fstart[fpos]
    lo = lo.copy()
    lo[fidx[forder]] = frank < flex_to_lo[fpos]

    order = np.lexsort((~lo, pos))
    dl_s, s_s, nv_s = dstloc[order], src[order], nv[order]
    pos_s, lo_s = pos[order], lo[order]
    gid = pos_s * 2 + (~lo_s).astype(np.int64)
    cnt = np.bincount(gid, minlength=npos * 2)
    assert (cnt[0::2] <= t_lo).all() and (cnt[1::2] <= t_hi).all()
    gstart = np.concatenate([[0], np.cumsum(cnt)[:-1]])
    rank = np.arange(dl_s.size) - gstart[gid]
    slot = np.where(lo_s, offL[pos_s] + rank, offH[pos_s] + rank)

    idx_lo = np.zeros(TL, np.int16)
    idx_hi = np.zeros(TH, np.int16)
    rowm = np.zeros((P, NCH), np.float32)
    nvm = np.zeros((P, NCH), np.float32)
    m = lo_s
    colm = cco[pos_s[m]] + (slot[m] >> 7) - csL[pos_s[m]]
    idx_lo[slot[m]] = s_s[m].astype(np.int16)
    rowm[slot[m] & (P - 1), colm] = dl_s[m].astype(np.float32)
    nvm[slot[m] & (P - 1), colm] = nv_s[m]
    m = ~lo_s
    colm = cco[pos_s[m]] + ncL[pos_s[m]] + (slot[m] >> 7) - csH[pos_s[m]]
    idx_hi[slot[m]] = (s_s[m] - HI_BASE).astype(np.int16)
    rowm[slot[m] & (P - 1), colm] = dl_s[m].astype(np.float32)
    nvm[slot[m] & (P - 1), colm] = nv_s[m]
    return _wrap(idx_lo), _wrap(idx_hi), rowm, nvm


def _build_stream2(pos, dstloc, src, nv, mt2):
    """Boundary-sharing single-stream tokens + meta for hop 2."""
    npos = mt2.size
    off, T2, cs, ce, ncols = _geom(mt2)
    cco = np.concatenate([[0], np.cumsum(ncols)[:-1]])
    NCH = int(ncols.sum())

    order = np.argsort(pos, kind="stable")
    pos_s, dl_s, s_s, nv_s = pos[order], dstloc[order], src[order], nv[order]
    cnt = np.bincount(pos_s, minlength=npos)
    assert (cnt <= mt2).all()
    gstart = np.concatenate([[0], np.cumsum(cnt)[:-1]])
    rank = np.arange(pos_s.size) - gstart[pos_s]
    slot = off[pos_s] + rank

    idx = np.zeros(T2, np.int16)
    idx[slot] = s_s.astype(np.int16)
    rowm = np.zeros((P, NCH), np.float32)
    nvm = np.zeros((P, NCH), np.float32)
    col = cco[pos_s] + (slot >> 7) - cs[pos_s]
    rowm[slot & (P - 1), col] = dl_s.astype(np.float32)
    nvm[slot & (P - 1), col] = nv_s
    return _wrap(idx), rowm, nvm


def _pack_positions(tot, blocks, npos):
    """Assign `blocks` to program positions by descending token count.
    Returns (order, inv) where order[p] = absolute block (-1 pad) and
    inv[blk] = position."""
    o = blocks[np.argsort(-tot[blocks], kind="stable")]
    order = np.full(npos, -1, np.int64)
    order[:o.size] = o
    inv = np.full(NB, -1, np.int64)
    inv[o] = np.arange(o.size)
    return order, inv


# ---------------- device program (SPMD over the 8 cores) ----------------

def _build_program(t_lo1, t_hi1, mt2):
    _, TLO1, csL1, ceL1, ncL1 = _geom(t_lo1)
    _, THI1, csH1, ceH1, ncH1 = _geom(t_hi1)
    cco1 = np.concatenate([[0], np.cumsum(ncL1 + ncH1)[:-1]])
    NCH1 = int((ncL1 + ncH1).sum())
    _, T2, cs2, ce2, nc2 = _geom(mt2)
    cco2 = np.concatenate([[0], np.cumsum(nc2)[:-1]])
    NCH2 = int(nc2.sum())
    nc = bacc.Bacc("TRN2", target_bir_lowering=False, debug=False, num_devices=1)
    x2 = nc.dram_tensor("x2", [NNP, TOKC], dt.bfloat16, kind="ExternalInput")
    w2_d = nc.dram_tensor("w2", [P, 2, P], dt.bfloat16, kind="ExternalInput")
    idx_d = {
        'lo': nc.dram_tensor("idx_lo", [P, TLO1 // 16], dt.int16, kind="ExternalInput"),
        'hi': nc.dram_tensor("idx_hi", [P, THI1 // 16], dt.int16, kind="ExternalInput"),
        'lo2': nc.dram_tensor("idx2", [P, T2 // 16], dt.int16, kind="ExternalInput"),
    }
    rowm1_d = nc.dram_tensor("rowm1", [P, NCH1], dt.float32, kind="ExternalInput")
    nvm1_d = nc.dram_tensor("nvm1", [P, NCH1], dt.float32, kind="ExternalInput")
    rowm2_d = nc.dram_tensor("rowm2", [P, NCH2], dt.float32, kind="ExternalInput")
    nvm2_d = nc.dram_tensor("nvm2", [P, NCH2], dt.float32, kind="ExternalInput")
    z2 = nc.dram_tensor("z2", [HALF_ROWS, TOKC], dt.bfloat16)
    outA = nc.dram_tensor("outA", [HALF_ROWS, TOKC], dt.bfloat16, kind="ExternalOutput")
    outB = nc.dram_tensor("outB", [NNP, TOKC], dt.bfloat16, kind="ExternalOutput")

    with tile.TileContext(nc) as tc:
        with (tc.tile_pool(name="const", bufs=1) as constp,
              tc.tile_pool(name="meta", bufs=1) as metap,
              tc.tile_pool(name="msg_lo", bufs=BUFS["msg_lo"]) as msglop,
              tc.tile_pool(name="msg_hi", bufs=BUFS["msg_hi"]) as msghip,
              tc.tile_pool(name="msg_lo2", bufs=BUFS["msg_lo2"]) as msglo2p,
              tc.tile_pool(name="idxp", bufs=BUFS["idxp"]) as idxp,
              tc.tile_pool(name="spp", bufs=BUFS["spp"]) as spp,
              tc.tile_pool(name="blkp", bufs=BUFS["blkp"]) as blkp,
              tc.tile_pool(name="psh", bufs=BUFS["psh"], space="PSUM") as psum_h,
              tc.tile_pool(name="pstr", bufs=BUFS["pstr"], space="PSUM") as psum_tr,
              tc.tile_pool(name="psout", bufs=BUFS["psout"], space="PSUM") as psum_out):

            iota_i = constp.tile([P, P], dt.int32)
            nc.gpsimd.iota(iota_i[:], pattern=[[1, P]], base=0, channel_multiplier=0)
            iota_f = constp.tile([P, P], dt.bfloat16)
            nc.vector.tensor_copy(iota_f[:], iota_i[:])
            ident = constp.tile([P, P], dt.bfloat16)
            make_identity(nc, ident[:])
            w2_sb = constp.tile([P, 2, P], dt.bfloat16)
            nc.sync.dma_start(out=w2_sb[:], in_=w2_d[:])
            rowm1_sb = metap.tile([P, NCH1], dt.float32)
            nc.sync.dma_start(out=rowm1_sb[:], in_=rowm1_d[:])
            nvm1_sb = metap.tile([P, NCH1], dt.float32)
            nc.sync.dma_start(out=nvm1_sb[:], in_=nvm1_d[:])
            rowm2_sb = metap.tile([P, NCH2], dt.float32)
            nc.sync.dma_start(out=rowm2_sb[:], in_=rowm2_d[:])
            nvm2_sb = metap.tile([P, NCH2], dt.float32)
            nc.sync.dma_start(out=nvm2_sb[:], in_=nvm2_d[:])

            slab_cache = {}

            def get_chunk(stream, src_ap, pool, T, gpos):
                tile_obj, s_cur = slab_cache.get(stream, (None, -1))
                s, j = divmod(gpos, GATHER_SLAB // P)
                if s != s_cur:
                    off = s * GATHER_SLAB
                    g = min(GATHER_SLAB, T - off)
                    it = idxp.tile([P, g // 16], dt.int16, tag="idx")
                    nc.sync.dma_start(
                        out=it[:], in_=idx_d[stream][:, off // 16:(off + g) // 16])
                    mt = pool.tile([P, g // P, TOKC], dt.bfloat16, tag="m" + stream)
                    nc.gpsimd.dma_gather(
                        out_ap=mt[:], in_ap=src_ap,
                        idxs_ap=it[:], num_idxs=g, num_idxs_reg=g,
                        elem_size=TOKC, single_packet=False)
                    slab_cache[stream] = (mt, s)
                    tile_obj = mt
                return tile_obj, j

            def build_sp(rowm_sb, nvm_sb, c):
                sp = spp.tile([P, P], dt.bfloat16, tag="sp")
                nc.vector.tensor_scalar(
                    sp[:], iota_f[:],
                    rowm_sb[:, c:c + 1], nvm_sb[:, c:c + 1],
                    mybir.AluOpType.is_equal, mybir.AluOpType.mult)
                return sp

            # ---- hop 1: h1[half] = (D^-1 A) h0;  outA = h1 @ W[0] and
            # z2 = h1 @ W[1] (W commutes with A: out = h1 W0 + A (h1 W1),
            # so hop 2 needs no W stage at all). ----
            for p in range(NPOS1):
                hp = psum_h.tile([P, 2, P], dt.float32, tag="hp")
                steps = [('lo', j, int(cco1[p]) + j - int(csL1[p]))
                         for j in range(int(csL1[p]), int(ceL1[p]) + 1)]
                if t_hi1[p] > 0:
                    steps += [('hi', j,
                               int(cco1[p] + ncL1[p]) + j - int(csH1[p]))
                              for j in range(int(csH1[p]), int(ceH1[p]) + 1)]
                for i, (stream, j, col) in enumerate(steps):
                    if stream == 'lo':
                        mt, jj = get_chunk('lo', x2[0:LO_LIMIT, :], msglop,
                                           TLO1, j)
                    else:
                        mt, jj = get_chunk('hi', x2[HI_BASE:NNP, :], msghip,
                                           THI1, j)
                    sp = build_sp(rowm1_sb, nvm1_sb, col)
                    nc.tensor.matmul(hp[:], sp[:], mt[:, jj, :],
                                     start=(i == 0), stop=(i == len(steps) - 1))
                h_sb = blkp.tile([P, 2, P], dt.bfloat16, tag="h_sb")
                nc.scalar.copy(h_sb[:], hp[:])
                tr = psum_tr.tile([P, 2, P], dt.bfloat16, tag="tr")
                nc.tensor.transpose(tr[:, 0, :], h_sb[:, 0, :], ident[:])
                nc.tensor.transpose(tr[:, 1, :], h_sb[:, 1, :], ident[:])
                trs = blkp.tile([P, 2, P], dt.bfloat16, tag="trs")
                nc.vector.tensor_copy(trs[:], tr[:])
                for k, dest in ((1, z2), (0, outA)):
                    op = psum_out.tile([P, 2, P], dt.float32, tag="op")
                    nc.tensor.matmul(op[:, 0, :], trs[:, 0, :], w2_sb[:, k, :],
                                     start=True, stop=True)
                    nc.tensor.matmul(op[:, 1, :], trs[:, 1, :], w2_sb[:, k, :],
                                     start=True, stop=True)
                    ob = blkp.tile([P, 2, P], dt.bfloat16, tag="ob")
                    if k == 1:
                        nc.scalar.copy(ob[:], op[:])
                    else:
                        nc.vector.tensor_copy(ob[:], op[:])
                    nc.sync.dma_start(out=dest[p * P:(p + 1) * P, :], in_=ob[:])

            # ---- hop 2: outB = (D^-1 A)|src-half z2 (final partial) ----
            # Boundary-sharing stream: chunk ranges [cs2[p], ce2[p]] overlap
            # between adjacent positions; each position has its own nv=0
            # masked meta column for a shared chunk.
            # The barrier orders hop-2's z2 gathers after hop-1's z2 writes
            # (DRAM RAW is not tracked at tile granularity).
            tc.strict_bb_all_engine_barrier()
            for p in range(NB):
                hp = psum_h.tile([P, 2, P], dt.float32, tag="hp")
                for j in range(int(cs2[p]), int(ce2[p]) + 1):
                    mt, jj = get_chunk('lo2', z2[0:HALF_ROWS, :], msglo2p, T2, j)
                    sp = build_sp(rowm2_sb, nvm2_sb,
                                  int(cco2[p]) + j - int(cs2[p]))
                    nc.tensor.matmul(hp[:], sp[:], mt[:, jj, :],
                                     start=(j == int(cs2[p])),
                                     stop=(j == int(ce2[p])))
                ob = blkp.tile([P, 2, P], dt.bfloat16, tag="ob")
                nc.scalar.copy(ob[:], hp[:])
                nc.sync.dma_start(out=outB[p * P:(p + 1) * P, :], in_=ob[:])

    nc.compile()
    return nc


# ---------------- entry point ----------------

def kernel(x, edge_index, edge_vals, W_f, W_b, bias):
    x = np.asarray(x, dtype=np.float32)
    edge_index = np.asarray(edge_index)
    edge_vals = np.asarray(edge_vals, dtype=np.float32)
    W_f = np.asarray(W_f, dtype=np.float32)
    W_b = np.asarray(W_b, dtype=np.float32)
    bias = np.asarray(bias, dtype=np.float32)

    rows = edge_index[0].astype(np.int64)
    cols = edge_index[1].astype(np.int64)
    deg = np.zeros(N_NODES, np.float32)
    np.add.at(deg, rows, edge_vals)
    deg += np.float32(1e-8)
    nv = (edge_vals / deg[rows]).astype(np.float32)

    halves = [np.arange(0, NPOS1), np.arange(NPOS1, NB)]
    v1 = []   # hop-1 variants: (pos, dstloc, src, nv, order)
    v2 = []   # hop-2 variants: (pos, dstloc, srcloc, nv, order, inv1)
    for d in range(2):
        dst, src = (rows, cols) if d == 0 else (cols, rows)
        dblk = dst >> 7
        dloc = dst & (P - 1)
        sblk = src >> 7
        tot1 = np.bincount(dblk, minlength=NB)
        tot2 = np.bincount(dblk, weights=(sblk >= NPOS1).astype(np.float64),
                           minlength=NB)
        for h in range(2):
            sel = (dblk >= NPOS1) == (h == 1)
            order1, inv1 = _pack_positions(tot1, halves[h], NPOS1)
            v1.append((inv1[dblk[sel]], dloc[sel], src[sel], nv[sel], order1))
            sel2 = (sblk >= NPOS1) == (h == 1)
            t2 = tot2 if h == 1 else (tot1 - tot2)
            order2, inv2 = _pack_positions(t2, np.arange(NB), NB)
            srcloc = inv1[sblk[sel2]] * P + (src[sel2] & (P - 1))
            v2.append((inv2[dblk[sel2]], dloc[sel2], srcloc, nv[sel2],
                       order2, inv1))

    c1 = [_classify(pos, src, NPOS1) for (pos, _, src, _, _) in v1]
    t_lo1, t_hi1, f2l1 = _choose_split(c1, NPOS1)
    mt2 = np.maximum.reduce([np.bincount(pos, minlength=NB)
                             for (pos, _, src, _, _, _) in v2])
    mt2 = np.maximum(mt2, 1)

    s1 = [_build_stream1(pos, dl, src, nvv, t_lo1, t_hi1, f2l1[i])
          for i, (pos, dl, src, nvv, _) in enumerate(v1)]
    s2 = [_build_stream2(pos, dl, src, nvv, mt2)
          for (pos, dl, src, nvv, _, _) in v2]

    key = (t_lo1.tobytes(), t_hi1.tobytes(), mt2.tobytes())
    if key not in _prog_cache:
        _prog_cache.clear()
        _prog_cache[key] = _build_program(t_lo1, t_hi1, mt2)
    nc = _prog_cache[key]

    in_maps = []
    for core in range(8):
        d, g, h = core >> 2, (core >> 1) & 1, core & 1
        vi = d * 2 + h
        Wd = W_f if d == 0 else W_b
        x2 = np.zeros((NNP, TOKC), bf16)
        x2[:N_NODES] = x[4 * g:4 * g + 4].transpose(1, 0, 2).reshape(
            N_NODES, TOKC).astype(bf16)
        w2 = np.zeros((P, 2, P), bf16)
        for k in range(2):
            for a in range(2):
                w2[C * a:C * a + C, k, C * a:C * a + C] = Wd[k].astype(bf16)
        in_maps.append({
            "x2": x2, "w2": w2,
            "idx_lo": s1[vi][0], "idx_hi": s1[vi][1],
            "rowm1": s1[vi][2], "nvm1": s1[vi][3],
            "idx2": s2[vi][0],
            "rowm2": s2[vi][1], "nvm2": s2[vi][2],
        })

    results = run_bass_kernel_spmd(nc, in_maps, list(range(8))).results

    out = np.empty((B, N_NODES, C), np.float32)
    for g in range(2):
        acc = np.zeros((NNP, TOKC), np.float32)
        for d in range(2):
            for h in range(2):
                vi = d * 2 + h
                r = results[(d << 2) | (g << 1) | h]
                order2 = v2[vi][4]
                inv2 = np.argsort(order2)
                acc += np.asarray(r["outB"]).astype(np.float32).reshape(
                    NB, P, TOKC)[inv2].reshape(NNP, TOKC)
                order1 = v1[vi][4]
                nreal = halves[h].size
                oa = np.asarray(r["outA"]).astype(np.float32).reshape(
                    NPOS1, P, TOKC)[:nreal]
                accb = acc.reshape(NB, P, TOKC)
                accb[order1[:nreal]] += oa
        for bl in range(4):
            out[4 * g + bl] = acc[:N_NODES, C * bl:C * bl + C]
    out += bias.reshape(1, 1, C)
    return out


# revision 30
# speedup vs baseline: 1.1603x; 1.0037x over previous
"""DiffusionGraphConv on 8 Trainium2 NeuronCores (Bass/Tile).

out = sum_k (D^-1 A)^k x W_f[k] + ((D^-1 A)^T)^k x W_b[k] + bias, K=2,
N=50000 nodes, E=800000 edges, B=8, C_in=C_out=64, f32.

Sharding: 8 cores = 2 diffusion directions x 2 batch-groups (4 batches
packed per 512B bf16 gather token) x 2 node-halves. No cross-core
traffic: hop 1 processes edges whose DESTINATION block falls in the
core's half (gathering from the replicated x), producing that half of
h1 = (D^-1 A) h0; hop 2 processes edges whose SOURCE falls in the same
half (gathering only from the core's own h1) and scatter-adds into all
destination blocks. The four partial outputs per batch-group (2 dirs x
2 halves) are summed on the host together with the bias.

Per hop on device: messages h[src[e]] are fetched with nc.gpsimd.dma_gather
(512B bf16 tokens); the scatter-add is a TensorE matmul per 128-edge chunk
with a one-hot matrix S[t,n] = (n == dst_local[t]) * nv[t] built by one DVE
tensor_scalar(is_equal, mult) op in bf16; chunks accumulate per 128-row
node block in PSUM. Because W commutes with the diffusion operator,
hop 1 emits both outA = h1 W[0] and the hop-2 gather source z2 = h1 W[1],
and hop 2's PSUM block is the final output partial (no trailing W stage).

Blocks are assigned to program positions per-core by descending token
count (bin-packing) so one SPMD program's per-position token counts,
taken as the max over the 4 edge-shard variants, waste little padding,
and token streams are continuous: a 128-token chunk straddling two
positions is matmul'ed once per position with complementary nv=0
masked one-hot columns (no per-position padding to chunk granularity).
"""
import numpy as np
import ml_dtypes

import concourse.bacc as bacc
import concourse.tile as tile
import concourse.mybir as mybir
from concourse.bass_utils import run_bass_kernel_spmd
from concourse.masks import make_identity

P = 128
N_NODES = 50000
N_EDGES = 800000
B, C = 8, 64
NNP = 50048          # nodes padded to a multiple of 128
NB = NNP // P        # 391 destination blocks
NPOS1 = 196          # hop-1 program positions (half0: 196 blocks, half1: 195)
HALF_ROWS = NPOS1 * P   # 25088 rows of h1 per core
LO_LIMIT = 32768     # src < LO_LIMIT -> lo gather stream (int16 idx range)
HI_BASE = NNP - 32768   # hi stream gathers rows [HI_BASE:], idx = src - HI_BASE
GATHER_SLAB = 1024   # tokens per dma_gather instruction
TOKC = 4 * C         # 256 bf16 values per token (4 batches x 64 ch) = 512B
dt = mybir.dt
bf16 = ml_dtypes.bfloat16

BUFS = dict(msg_lo=10, msg_hi=5, msg_lo2=10, idxp=8, spp=16, blkp=7,
            psh=2, pstr=2, psout=3)

_prog_cache = {}


# ---------------- host-side prep ----------------

def _classify(pos, src, npos):
    """Per-position (must-lo, must-hi, flexible) source counts."""
    ml = np.bincount(pos[src < HI_BASE], minlength=npos)
    mh = np.bincount(pos[src >= LO_LIMIT], minlength=npos)
    fx = np.bincount(pos[(src >= HI_BASE) & (src < LO_LIMIT)], minlength=npos)
    return ml, mh, fx


def _choose_split(cands, npos):
    """Per-position unified lo/hi token counts (t_lo, t_hi) covering every
    variant in `cands` ((ml, mh, fx) triples), minimizing t_lo + t_hi via a
    scan over flex thresholds; plus each variant's flex-to-lo counts."""
    nv_ = len(cands)
    t_lo = np.zeros(npos, np.int64)
    t_hi = np.zeros(npos, np.int64)
    f2l = [np.zeros(npos, np.int64) for _ in range(nv_)]
    for p in range(npos):
        mls = [int(ml[p]) for (ml, _, _) in cands]
        fxs = [int(fx[p]) for (_, _, fx) in cands]
        tots = [int((ml + mh + fx)[p]) for (ml, mh, fx) in cands]
        best = None
        for T in sorted({m for m in mls} | {m + f for m, f in zip(mls, fxs)}):
            lo = [min(max(T, m), m + f) for m, f in zip(mls, fxs)]
            hi = [t - lv for t, lv in zip(tots, lo)]
            cost = max(lo) + max(hi)
            if best is None or cost < best[0]:
                best = (cost, max(lo), max(hi), lo)
        t_lo[p] = max(best[1], 1)
        t_hi[p] = best[2]
        for v in range(nv_):
            f2l[v][p] = best[3][v] - mls[v]
    return t_lo, t_hi, f2l


def _geom(t):
    """Continuous-stream geometry for per-position token counts `t`:
    (offsets, padded length, first chunk, last chunk, meta columns)."""
    off = np.concatenate([[0], np.cumsum(t)[:-1]])
    T = int(((int(t.sum()) + P - 1) // P) * P)
    cs = off >> 7
    ce = (off + t - 1) >> 7
    ncols = np.where(t > 0, ce - cs + 1, 0)
    return off, T, cs, ce, ncols


def _wrap(a):  # [T] -> [128, T/16]; token i at [i%16, i//16], replicated 8x
    if a.size == 0:
        return np.zeros((P, 0), np.int16)
    return np.ascontiguousarray(np.tile(a.reshape(a.size // 16, 16).T, (8, 1)))


def _build_stream1(pos, dstloc, src, nv, t_lo, t_hi, flex_to_lo):
    """Boundary-sharing lo/hi token streams + chunk-major meta for one
    hop-1 shard variant. Position p's lo tokens occupy lo-stream slots
    [offL[p], offL[p]+t_lo[p]), hi tokens likewise; chunks shared between
    adjacent positions appear in both positions' meta columns with
    complementary nv=0 masking. Padding tokens: idx 0 / nv 0 / dst 0."""
    npos = t_lo.size
    offL, TL, csL, ceL, ncL = _geom(t_lo)
    offH, TH, csH, ceH, ncH = _geom(t_hi)
    cco = np.concatenate([[0], np.cumsum(ncL + ncH)[:-1]])
    NCH = int((ncL + ncH).sum())

    lo = src < HI_BASE
    flex = (src >= HI_BASE) & (src < LO_LIMIT)
    fidx = np.flatnonzero(flex)
    forder = np.argsort(pos[fidx], kind="stable")
    fpos = pos[fidx[forder]]
    fcnt = np.bincount(fpos, minlength=npos)
    fstart = np.concatenate([[0], np.cumsum(fcnt)[:-1]])
    frank = np.arange(fidx.size) - fstart[fpos]
    lo = lo.copy()
    lo[fidx[forder]] = frank < flex_to_lo[fpos]

    order = np.lexsort((~lo, pos))
    dl_s, s_s, nv_s = dstloc[order], src[order], nv[order]
    pos_s, lo_s = pos[order], lo[order]
    gid = pos_s * 2 + (~lo_s).astype(np.int64)
    cnt = np.bincount(gid, minlength=npos * 2)
    assert (cnt[0::2] <= t_lo).all() and (cnt[1::2] <= t_hi).all()
    gstart = np.concatenate([[0], np.cumsum(cnt)[:-1]])
    rank = np.arange(dl_s.size) - gstart[gid]
    slot = np.where(lo_s, offL[pos_s] + rank, offH[pos_s] + rank)

    idx_lo = np.zeros(TL, np.int16)
    idx_hi = np.zeros(TH, np.int16)
    rowm = np.zeros((P, NCH), np.float32)
    nvm = np.zeros((P, NCH), np.float32)
    m = lo_s
    colm = cco[pos_s[m]] + (slot[m] >> 7) - csL[pos_s[m]]
    idx_lo[slot[m]] = s_s[m].astype(np.int16)
    rowm[slot[m] & (P - 1), colm] = dl_s[m].astype(np.float32)
    nvm[slot[m] & (P - 1), colm] = nv_s[m]
    m = ~lo_s
    colm = cco[pos_s[m]] + ncL[pos_s[m]] + (slot[m] >> 7) - csH[pos_s[m]]
    idx_hi[slot[m]] = (s_s[m] - HI_BASE).astype(np.int16)
    rowm[slot[m] & (P - 1), colm] = dl_s[m].astype(np.float32)
    nvm[slot[m] & (P - 1), colm] = nv_s[m]
    return _wrap(idx_lo), _wrap(idx_hi), rowm, nvm


def _build_stream2(pos, dstloc, src, nv, mt2):
    """Boundary-sharing single-stream tokens + meta for hop 2."""
    npos = mt2.size
    off, T2, cs, ce, ncols = _geom(mt2)
    cco = np.concatenate([[0], np.cumsum(ncols)[:-1]])
    NCH = int(ncols.sum())

    order = np.argsort(pos, kind="stable")
    pos_s, dl_s, s_s, nv_s = pos[order], dstloc[order], src[order], nv[order]
    cnt = np.bincount(pos_s, minlength=npos)
    assert (cnt <= mt2).all()
    gstart = np.concatenate([[0], np.cumsum(cnt)[:-1]])
    rank = np.arange(pos_s.size) - gstart[pos_s]
    slot = off[pos_s] + rank

    idx = np.zeros(T2, np.int16)
    idx[slot] = s_s.astype(np.int16)
    rowm = np.zeros((P, NCH), np.float32)
    nvm = np.zeros((P, NCH), np.float32)
    col = cco[pos_s] + (slot >> 7) - cs[pos_s]
    rowm[slot & (P - 1), col] = dl_s.astype(np.float32)
    nvm[slot & (P - 1), col] = nv_s
    return _wrap(idx), rowm, nvm


def _pack_positions(tot, blocks, npos):
    """Assign `blocks` to program positions by descending token count.
    Returns (order, inv) where order[p] = absolute block (-1 pad) and
    inv[blk] = position."""
    o = blocks[np.argsort(-tot[blocks], kind="stable")]
    order = np.full(npos, -1, np.int64)
    order[:o.size] = o
    inv = np.full(NB, -1, np.int64)
    inv[o] = np.arange(o.size)
    return order, inv


# ---------------- device program (SPMD over the 8 cores) ----------------

def _build_program(t_lo1, t_hi1, mt2):
    _, TLO1, csL1, ceL1, ncL1 = _geom(t_lo1)
    _, THI1, csH1, ceH1, ncH1 = _geom(t_hi1)
    cco1 = np.concatenate([[0], np.cumsum(ncL1 + ncH1)[:-1]])
    NCH1 = int((ncL1 + ncH1).sum())
    _, T2, cs2, ce2, nc2 = _geom(mt2)
    cco2 = np.concatenate([[0], np.cumsum(nc2)[:-1]])
    NCH2 = int(nc2.sum())
    nc = bacc.Bacc("TRN2", target_bir_lowering=False, debug=False, num_devices=1)
    x2 = nc.dram_tensor("x2", [NNP, TOKC], dt.bfloat16, kind="ExternalInput")
    w2_d = nc.dram_tensor("w2", [P, 2, P], dt.bfloat16, kind="ExternalInput")
    idx_d = {
        'lo': nc.dram_tensor("idx_lo", [P, TLO1 // 16], dt.int16, kind="ExternalInput"),
        'hi': nc.dram_tensor("idx_hi", [P, THI1 // 16], dt.int16, kind="ExternalInput"),
        'lo2': nc.dram_tensor("idx2", [P, T2 // 16], dt.int16, kind="ExternalInput"),
    }
    rowm1_d = nc.dram_tensor("rowm1", [P, NCH1], dt.float32, kind="ExternalInput")
    nvm1_d = nc.dram_tensor("nvm1", [P, NCH1], dt.float32, kind="ExternalInput")
    rowm2_d = nc.dram_tensor("rowm2", [P, NCH2], dt.float32, kind="ExternalInput")
    nvm2_d = nc.dram_tensor("nvm2", [P, NCH2], dt.float32, kind="ExternalInput")
    z2 = nc.dram_tensor("z2", [HALF_ROWS, TOKC], dt.bfloat16)
    outA = nc.dram_tensor("outA", [HALF_ROWS, TOKC], dt.bfloat16, kind="ExternalOutput")
    outB = nc.dram_tensor("outB", [NNP, TOKC], dt.bfloat16, kind="ExternalOutput")

    with tile.TileContext(nc) as tc:
        with (tc.tile_pool(name="const", bufs=1) as constp,
              tc.tile_pool(name="meta", bufs=1) as metap,
              tc.tile_pool(name="msg_lo", bufs=BUFS["msg_lo"]) as msglop,
              tc.tile_pool(name="msg_hi", bufs=BUFS["msg_hi"]) as msghip,
              tc.tile_pool(name="msg_lo2", bufs=BUFS["msg_lo2"]) as msglo2p,
              tc.tile_pool(name="idxp", bufs=BUFS["idxp"]) as idxp,
              tc.tile_pool(name="spp", bufs=BUFS["spp"]) as spp,
              tc.tile_pool(name="blkp", bufs=BUFS["blkp"]) as blkp,
              tc.tile_pool(name="psh", bufs=BUFS["psh"], space="PSUM") as psum_h,
              tc.tile_pool(name="pstr", bufs=BUFS["pstr"], space="PSUM") as psum_tr,
              tc.tile_pool(name="psout", bufs=BUFS["psout"], space="PSUM") as psum_out):

            iota_i = constp.tile([P, P], dt.int32)
            nc.gpsimd.iota(iota_i[:], pattern=[[1, P]], base=0, channel_multiplier=0)
            iota_f = constp.tile([P, P], dt.bfloat16)
            nc.vector.tensor_copy(iota_f[:], iota_i[:])
            ident = constp.tile([P, P], dt.bfloat16)
            make_identity(nc, ident[:])
            w2_sb = constp.tile([P, 2, P], dt.bfloat16)
            nc.sync.dma_start(out=w2_sb[:], in_=w2_d[:])
            rowm1_sb = metap.tile([P, NCH1], dt.float32)
            nc.sync.dma_start(out=rowm1_sb[:], in_=rowm1_d[:])
            nvm1_sb = metap.tile([P, NCH1], dt.float32)
            nc.sync.dma_start(out=nvm1_sb[:], in_=nvm1_d[:])
            rowm2_sb = metap.tile([P, NCH2], dt.float32)
            nc.sync.dma_start(out=rowm2_sb[:], in_=rowm2_d[:])
            nvm2_sb = metap.tile([P, NCH2], dt.float32)
            nc.sync.dma_start(out=nvm2_sb[:], in_=nvm2_d[:])

            slab_cache = {}

            def get_chunk(stream, src_ap, pool, T, gpos):
                tile_obj, s_cur = slab_cache.get(stream, (None, -1))
                s, j = divmod(gpos, GATHER_SLAB // P)
                if s != s_cur:
                    off = s * GATHER_SLAB
                    g = min(GATHER_SLAB, T - off)
                    it = idxp.tile([P, g // 16], dt.int16, tag="idx")
                    nc.sync.dma_start(
                        out=it[:], in_=idx_d[stream][:, off // 16:(off + g) // 16])
                    mt = pool.tile([P, g // P, TOKC], dt.bfloat16, tag="m" + stream)
                    nc.gpsimd.dma_gather(
                        out_ap=mt[:], in_ap=src_ap,
                        idxs_ap=it[:], num_idxs=g, num_idxs_reg=g,
                        elem_size=TOKC, single_packet=False)
                    slab_cache[stream] = (mt, s)
                    tile_obj = mt
                return tile_obj, j

            def build_sp(rowm_sb, nvm_sb, c):
                sp = spp.tile([P, P], dt.bfloat16, tag="sp")
                nc.vector.tensor_scalar(
                    sp[:], iota_f[:],
                    rowm_sb[:, c:c + 1], nvm_sb[:, c:c + 1],
                    mybir.AluOpType.is_equal, mybir.AluOpType.mult)
                return sp

            # ---- hop 1: h1[half] = (D^-1 A) h0;  outA = h1 @ W[0] and
            # z2 = h1 @ W[1] (W commutes with A: out = h1 W0 + A (h1 W1),
            # so hop 2 needs no W stage at all). ----
            for p in range(NPOS1):
                hp = psum_h.tile([P, 2, P], dt.float32, tag="hp")
                steps = [('lo', j, int(cco1[p]) + j - int(csL1[p]))
                         for j in range(int(csL1[p]), int(ceL1[p]) + 1)]
                if t_hi1[p] > 0:
                    steps += [('hi', j,
                               int(cco1[p] + ncL1[p]) + j - int(csH1[p]))
                              for j in range(int(csH1[p]), int(ceH1[p]) + 1)]
                for i, (stream, j, col) in enumerate(steps):
                    if stream == 'lo':
                        mt, jj = get_chunk('lo', x2[0:LO_LIMIT, :], msglop,
                                           TLO1, j)
                    else:
                        mt, jj = get_chunk('hi', x2[HI_BASE:NNP, :], msghip,
                                           THI1, j)
                    sp = build_sp(rowm1_sb, nvm1_sb, col)
                    nc.tensor.matmul(hp[:], sp[:], mt[:, jj, :],
                                     start=(i == 0), stop=(i == len(steps) - 1))
                h_sb = blkp.tile([P, 2, P], dt.bfloat16, tag="h_sb")
                nc.scalar.copy(h_sb[:], hp[:])
                tr = psum_tr.tile([P, 2, P], dt.bfloat16, tag="tr")
                nc.tensor.transpose(tr[:, 0, :], h_sb[:, 0, :], ident[:])
                nc.tensor.transpose(tr[:, 1, :], h_sb[:, 1, :], ident[:])
                trs = blkp.tile([P, 2, P], dt.bfloat16, tag="trs")
                nc.vector.tensor_copy(trs[:], tr[:])
                for k, dest in ((1, z2), (0, outA)):
                    op = psum_out.tile([P, 2, P], dt.float32, tag="op")
                    nc.tensor.matmul(op[:, 0, :], trs[:, 0, :], w2_sb[:, k, :],
                                     start=True, stop=True)
                    nc.tensor.matmul(op[:, 1, :], trs[:, 1, :], w2_sb[:, k, :],
                                     start=True, stop=True)
                    ob = blkp.tile([P, 2, P], dt.bfloat16, tag="ob")
                    if k == 1:
                        nc.scalar.copy(ob[:], op[:])
                    else:
                        nc.vector.tensor_copy(ob[:], op[:])
                    nc.sync.dma_start(out=dest[p * P:(p + 1) * P, :], in_=ob[:])

            # ---- hop 2: outB = (D^-1 A)|src-half z2 (final partial) ----
            # Boundary-sharing stream: chunk ranges [cs2[p], ce2[p]] overlap
            # between adjacent positions; each position has its own nv=0
            # masked meta column for a shared chunk.
            # The barrier orders hop-2's z2 gathers after hop-1's z2 writes
            # (DRAM RAW is not tracked at tile granularity).
            tc.strict_bb_all_engine_barrier()
            for p in range(NB):
                hp = psum_h.tile([P, 2, P], dt.float32, tag="hp")
                for j in range(int(cs2[p]), int(ce2[p]) + 1):
                    mt, jj = get_chunk('lo2', z2[0:HALF_ROWS, :], msglo2p, T2, j)
                    sp = build_sp(rowm2_sb, nvm2_sb,
                                  int(cco2[p]) + j - int(cs2[p]))
                    nc.tensor.matmul(hp[:], sp[:], mt[:, jj, :],
                                     start=(j == int(cs2[p])),
                                     stop=(j == int(ce2[p])))
                ob = blkp.tile([P, 2, P], dt.bfloat16, tag="ob")
                nc.scalar.copy(ob[:], hp[:])
                nc.sync.dma_start(out=outB[p * P:(p + 1) * P, :], in_=ob[:])

    nc.compile()
    return nc


# ---------------- entry point ----------------

def kernel(x, edge_index, edge_vals, W_f, W_b, bias):
    x = np.asarray(x, dtype=np.float32)
    edge_index = np.asarray(edge_index)
    edge_vals = np.asarray(edge_vals, dtype=np.float32)
    W_f = np.asarray(W_f, dtype=np.float32)
    W_b = np.asarray(W_b, dtype=np.float32)
    bias = np.asarray(bias, dtype=np.float32)

    rows = edge_index[0].astype(np.int64)
    cols = edge_index[1].astype(np.int64)
    deg = np.zeros(N_NODES, np.float32)
    np.add.at(deg, rows, edge_vals)
    deg += np.float32(1e-8)
    nv = (edge_vals / deg[rows]).astype(np.float32)

    halves = [np.arange(0, NPOS1), np.arange(NPOS1, NB)]
    v1 = []   # hop-1 variants: (pos, dstloc, src, nv, order)
    v2 = []   # hop-2 variants: (pos, dstloc, srcloc, nv, order, inv1)
    for d in range(2):
        dst, src = (rows, cols) if d == 0 else (cols, rows)
        dblk = dst >> 7
        dloc = dst & (P - 1)
        sblk = src >> 7
        tot1 = np.bincount(dblk, minlength=NB)
        tot2 = np.bincount(dblk, weights=(sblk >= NPOS1).astype(np.float64),
                           minlength=NB)
        for h in range(2):
            sel = (dblk >= NPOS1) == (h == 1)
            order1, inv1 = _pack_positions(tot1, halves[h], NPOS1)
            v1.append((inv1[dblk[sel]], dloc[sel], src[sel], nv[sel], order1))
            sel2 = (sblk >= NPOS1) == (h == 1)
            t2 = tot2 if h == 1 else (tot1 - tot2)
            order2, inv2 = _pack_positions(t2, np.arange(NB), NB)
            srcloc = inv1[sblk[sel2]] * P + (src[sel2] & (P - 1))
            v2.append((inv2[dblk[sel2]], dloc[sel2], srcloc, nv[sel2],
                       order2, inv1))

    c1 = [_classify(pos, src, NPOS1) for (pos, _, src, _, _) in v1]
    t_lo1, t_hi1, f2l1 = _choose_split(c1, NPOS1)
    mt2 = np.maximum.reduce([np.bincount(pos, minlength=NB)
                             for (pos, _, src, _, _, _) in v2])
    mt2 = np.maximum(mt2, 1)

    s1 = [_build_stream1(pos, dl, src, nvv, t_lo1, t_hi1, f2l1[i])
          for i, (pos, dl, src, nvv, _) in enumerate(v1)]
    s2 = [_build_stream2(pos, dl, src, nvv, mt2)
          for (pos, dl, src, nvv, _, _) in v2]

    key = (t_lo1.tobytes(), t_hi1.tobytes(), mt2.tobytes())
    if key not in _prog_cache:
        _prog_cache.clear()
        _prog_cache[key] = _build_program(t_lo1, t_hi1, mt2)
    nc = _prog_cache[key]

    in_maps = []
    for core in range(8):
        d, g, h = core >> 2, (core >> 1) & 1, core & 1
        vi = d * 2 + h
        Wd = W_f if d == 0 else W_b
        x2 = np.zeros((NNP, TOKC), bf16)
        x2[:N_NODES] = x[4 * g:4 * g + 4].transpose(1, 0, 2).reshape(
            N_NODES, TOKC).astype(bf16)
        w2 = np.zeros((P, 2, P), bf16)
        for k in range(2):
            for a in range(2):
                w2[C * a:C * a + C, k, C * a:C * a + C] = Wd[k].astype(bf16)
        in_maps.append({
            "x2": x2, "w2": w2,
            "idx_lo": s1[vi][0], "idx_hi": s1[vi][1],
            "rowm1": s1[vi][2], "nvm1": s1[vi][3],
            "idx2": s2[vi][0],
            "rowm2": s2[vi][1], "nvm2": s2[vi][2],
        })

    results = run_bass_kernel_spmd(nc, in_maps, list(range(8))).results

    out = np.empty((B, N_NODES, C), np.float32)
    for g in range(2):
        acc = np.zeros((NNP, TOKC), np.float32)
        for d in range(2):
            for h in range(2):
                vi = d * 2 + h
                r = results[(d << 2) | (g << 1) | h]
                order2 = v2[vi][4]
                inv2 = np.argsort(order2)
                acc += np.asarray(r["outB"]).astype(np.float32).reshape(
                    NB, P, TOKC)[inv2].reshape(NNP, TOKC)
                order1 = v1[vi][4]
                nreal = halves[h].size
                oa = np.asarray(r["outA"]).astype(np.float32).reshape(
                    NPOS1, P, TOKC)[:nreal]
                accb = acc.reshape(NB, P, TOKC)
                accb[order1[:nreal]] += oa
        for bl in range(4):
            out[4 * g + bl] = acc[:N_NODES, C * bl:C * bl + C]
    out += bias.reshape(1, 1, C)
    return out


# revision 33
# speedup vs baseline: 1.1798x; 1.0168x over previous
"""DiffusionGraphConv on 8 Trainium2 NeuronCores (Bass/Tile).

out = sum_k (D^-1 A)^k x W_f[k] + ((D^-1 A)^T)^k x W_b[k] + bias, K=2,
N=50000 nodes, E=800000 edges, B=8, C_in=C_out=64, f32.

Sharding: 8 cores = 2 diffusion directions x 2 batch-groups (4 batches
packed per 512B bf16 gather token) x 2 node-halves. No cross-core
traffic: hop 1 processes edges whose DESTINATION block falls in the
core's half (gathering from the replicated x), producing that half of
h1 = (D^-1 A) h0; hop 2 processes edges whose SOURCE falls in the same
half (gathering only from the core's own h1) and scatter-adds into all
destination blocks. The four partial outputs per batch-group (2 dirs x
2 halves) are summed on the host together with the bias.

Per hop on device: messages h[src[e]] are fetched with nc.gpsimd.dma_gather
(512B bf16 tokens); the scatter-add is a TensorE matmul per 128-edge chunk
with a one-hot matrix S[t,n] = (n == dst_local[t]) * nv[t] built by one DVE
tensor_scalar(is_equal, mult) op in bf16; chunks accumulate per 128-row
node block in PSUM. Because W commutes with the diffusion operator,
hop 1 emits both outA = h1 W[0] and the hop-2 gather source z2 = h1 W[1],
and hop 2's PSUM block is the final output partial (no trailing W stage).

Blocks are assigned to program positions per-core by descending token
count (bin-packing) so one SPMD program's per-position token counts,
taken as the max over the 4 edge-shard variants, waste little padding,
and token streams are continuous: a 128-token chunk straddling two
positions is matmul'ed once per position with complementary nv=0
masked one-hot columns (no per-position padding to chunk granularity).
"""
import numpy as np
import ml_dtypes

import concourse.bacc as bacc
import concourse.tile as tile
import concourse.mybir as mybir
from concourse.bass_utils import run_bass_kernel_spmd
from concourse.masks import make_identity

P = 128
N_NODES = 50000
N_EDGES = 800000
B, C = 8, 64
NNP = 50048          # nodes padded to a multiple of 128
NB = NNP // P        # 391 destination blocks
NPOS1 = 196          # hop-1 program positions (half0: 196 blocks, half1: 195)
HALF_ROWS = NPOS1 * P   # 25088 rows of h1 per core
LO_LIMIT = 32768     # src < LO_LIMIT -> lo gather stream (int16 idx range)
HI_BASE = NNP - 32768   # hi stream gathers rows [HI_BASE:], idx = src - HI_BASE
GATHER_SLAB = 1024   # tokens per dma_gather instruction
TOKC = 4 * C         # 256 bf16 values per token (4 batches x 64 ch) = 512B
dt = mybir.dt
bf16 = ml_dtypes.bfloat16

BUFS = dict(msg_lo=10, msg_hi=5, msg_lo2=10, idxp=8, spp=16, blkp=7,
            psh=2, pstr=2, psout=3)

_prog_cache = {}


# ---------------- host-side prep ----------------

def _classify(pos, src, npos):
    """Per-position (must-lo, must-hi, flexible) source counts."""
    ml = np.bincount(pos[src < HI_BASE], minlength=npos)
    mh = np.bincount(pos[src >= LO_LIMIT], minlength=npos)
    fx = np.bincount(pos[(src >= HI_BASE) & (src < LO_LIMIT)], minlength=npos)
    return ml, mh, fx


def _choose_split(cands, npos):
    """Per-position unified lo/hi token counts (t_lo, t_hi) covering every
    variant in `cands` ((ml, mh, fx) triples), minimizing t_lo + t_hi via a
    scan over flex thresholds; plus each variant's flex-to-lo counts."""
    nv_ = len(cands)
    t_lo = np.zeros(npos, np.int64)
    t_hi = np.zeros(npos, np.int64)
    f2l = [np.zeros(npos, np.int64) for _ in range(nv_)]
    for p in range(npos):
        mls = [int(ml[p]) for (ml, _, _) in cands]
        fxs = [int(fx[p]) for (_, _, fx) in cands]
        tots = [int((ml + mh + fx)[p]) for (ml, mh, fx) in cands]
        best = None
        for T in sorted({m for m in mls} | {m + f for m, f in zip(mls, fxs)}):
            lo = [min(max(T, m), m + f) for m, f in zip(mls, fxs)]
            hi = [t - lv for t, lv in zip(tots, lo)]
            cost = max(lo) + max(hi)
            if best is None or cost < best[0]:
                best = (cost, max(lo), max(hi), lo)
        t_lo[p] = max(best[1], 1)
        t_hi[p] = best[2]
        for v in range(nv_):
            f2l[v][p] = best[3][v] - mls[v]
    return t_lo, t_hi, f2l


def _geom(t):
    """Continuous-stream geometry for per-position token counts `t`:
    (offsets, padded length, first chunk, last chunk, meta columns)."""
    off = np.concatenate([[0], np.cumsum(t)[:-1]])
    T = int(((int(t.sum()) + P - 1) // P) * P)
    cs = off >> 7
    ce = (off + t - 1) >> 7
    ncols = np.where(t > 0, ce - cs + 1, 0)
    return off, T, cs, ce, ncols


def _wrap(a):  # [T] -> [128, T/16]; token i at [i%16, i//16], replicated 8x
    if a.size == 0:
        return np.zeros((P, 0), np.int16)
    return np.ascontiguousarray(np.tile(a.reshape(a.size // 16, 16).T, (8, 1)))


def _build_stream1(pos, dstloc, src, nv, t_lo, t_hi, flex_to_lo):
    """Boundary-sharing lo/hi token streams + chunk-major meta for one
    hop-1 shard variant. Position p's lo tokens occupy lo-stream slots
    [offL[p], offL[p]+t_lo[p]), hi tokens likewise; chunks shared between
    adjacent positions appear in both positions' meta columns with
    complementary nv=0 masking. Padding tokens: idx 0 / nv 0 / dst 0."""
    npos = t_lo.size
    offL, TL, csL, ceL, ncL = _geom(t_lo)
    offH, TH, csH, ceH, ncH = _geom(t_hi)
    cco = np.concatenate([[0], np.cumsum(ncL + ncH)[:-1]])
    NCH = int((ncL + ncH).sum())

    lo = src < HI_BASE
    flex = (src >= HI_BASE) & (src < LO_LIMIT)
    fidx = np.flatnonzero(flex)
    forder = np.argsort(pos[fidx], kind="stable")
    fpos = pos[fidx[forder]]
    fcnt = np.bincount(fpos, minlength=npos)
    fstart = np.concatenate([[0], np.cumsum(fcnt)[:-1]])
    frank = np.arange(fidx.size) - fstart[fpos]
    lo = lo.copy()
    lo[fidx[forder]] = frank < flex_to_lo[fpos]

    order = np.lexsort((~lo, pos))
    dl_s, s_s, nv_s = dstloc[order], src[order], nv[order]
    pos_s, lo_s = pos[order], lo[order]
    gid = pos_s * 2 + (~lo_s).astype(np.int64)
    cnt = np.bincount(gid, minlength=npos * 2)
    assert (cnt[0::2] <= t_lo).all() and (cnt[1::2] <= t_hi).all()
    gstart = np.concatenate([[0], np.cumsum(cnt)[:-1]])
    rank = np.arange(dl_s.size) - gstart[gid]
    slot = np.where(lo_s, offL[pos_s] + rank, offH[pos_s] + rank)

    idx_lo = np.zeros(TL, np.int16)
    idx_hi = np.zeros(TH, np.int16)
    rowm = np.zeros((P, NCH), np.float32)
    nvm = np.zeros((P, NCH), np.float32)
    m = lo_s
    colm = cco[pos_s[m]] + (slot[m] >> 7) - csL[pos_s[m]]
    idx_lo[slot[m]] = s_s[m].astype(np.int16)
    rowm[slot[m] & (P - 1), colm] = dl_s[m].astype(np.float32)
    nvm[slot[m] & (P - 1), colm] = nv_s[m]
    m = ~lo_s
    colm = cco[pos_s[m]] + ncL[pos_s[m]] + (slot[m] >> 7) - csH[pos_s[m]]
    idx_hi[slot[m]] = (s_s[m] - HI_BASE).astype(np.int16)
    rowm[slot[m] & (P - 1), colm] = dl_s[m].astype(np.float32)
    nvm[slot[m] & (P - 1), colm] = nv_s[m]
    return _wrap(idx_lo), _wrap(idx_hi), rowm, nvm


def _build_stream2(pos, dstloc, src, nv, mt2):
    """Boundary-sharing single-stream tokens + meta for hop 2."""
    npos = mt2.size
    off, T2, cs, ce, ncols = _geom(mt2)
    cco = np.concatenate([[0], np.cumsum(ncols)[:-1]])
    NCH = int(ncols.sum())

    order = np.argsort(pos, kind="stable")
    pos_s, dl_s, s_s, nv_s = pos[order], dstloc[order], src[order], nv[order]
    cnt = np.bincount(pos_s, minlength=npos)
    assert (cnt <= mt2).all()
    gstart = np.concatenate([[0], np.cumsum(cnt)[:-1]])
    rank = np.arange(pos_s.size) - gstart[pos_s]
    slot = off[pos_s] + rank

    idx = np.zeros(T2, np.int16)
    idx[slot] = s_s.astype(np.int16)
    rowm = np.zeros((P, NCH), np.float32)
    nvm = np.zeros((P, NCH), np.float32)
    col = cco[pos_s] + (slot >> 7) - cs[pos_s]
    rowm[slot & (P - 1), col] = dl_s.astype(np.float32)
    nvm[slot & (P - 1), col] = nv_s
    return _wrap(idx), rowm, nvm


def _pack_positions(tot, blocks, npos):
    """Assign `blocks` to program positions by descending token count.
    Returns (order, inv) where order[p] = absolute block (-1 pad) and
    inv[blk] = position."""
    o = blocks[np.argsort(-tot[blocks], kind="stable")]
    order = np.full(npos, -1, np.int64)
    order[:o.size] = o
    inv = np.full(NB, -1, np.int64)
    inv[o] = np.arange(o.size)
    return order, inv


# ---------------- device program (SPMD over the 8 cores) ----------------

def _build_program(t_lo1, t_hi1, mt2):
    _, TLO1, csL1, ceL1, ncL1 = _geom(t_lo1)
    _, THI1, csH1, ceH1, ncH1 = _geom(t_hi1)
    cco1 = np.concatenate([[0], np.cumsum(ncL1 + ncH1)[:-1]])
    NCH1 = int((ncL1 + ncH1).sum())
    _, T2, cs2, ce2, nc2 = _geom(mt2)
    cco2 = np.concatenate([[0], np.cumsum(nc2)[:-1]])
    NCH2 = int(nc2.sum())
    nc = bacc.Bacc("TRN2", target_bir_lowering=False, debug=False, num_devices=1)
    x2 = nc.dram_tensor("x2", [NNP, TOKC], dt.bfloat16, kind="ExternalInput")
    w2_d = nc.dram_tensor("w2", [P, 2, P], dt.bfloat16, kind="ExternalInput")
    idx_d = {
        'lo': nc.dram_tensor("idx_lo", [P, TLO1 // 16], dt.int16, kind="ExternalInput"),
        'hi': nc.dram_tensor("idx_hi", [P, THI1 // 16], dt.int16, kind="ExternalInput"),
        'lo2': nc.dram_tensor("idx2", [P, T2 // 16], dt.int16, kind="ExternalInput"),
    }
    rowm1_d = nc.dram_tensor("rowm1", [P, NCH1], dt.float32, kind="ExternalInput")
    nvm1_d = nc.dram_tensor("nvm1", [P, NCH1], dt.float32, kind="ExternalInput")
    rowm2_d = nc.dram_tensor("rowm2", [P, NCH2], dt.float32, kind="ExternalInput")
    nvm2_d = nc.dram_tensor("nvm2", [P, NCH2], dt.float32, kind="ExternalInput")
    z2 = nc.dram_tensor("z2", [HALF_ROWS, TOKC], dt.bfloat16)
    outA = nc.dram_tensor("outA", [HALF_ROWS, TOKC], dt.bfloat16, kind="ExternalOutput")
    outB = nc.dram_tensor("outB", [NNP, TOKC], dt.bfloat16, kind="ExternalOutput")

    with tile.TileContext(nc) as tc:
        with (tc.tile_pool(name="const", bufs=1) as constp,
              tc.tile_pool(name="meta", bufs=1) as metap,
              tc.tile_pool(name="msg_lo", bufs=BUFS["msg_lo"]) as msglop,
              tc.tile_pool(name="msg_hi", bufs=BUFS["msg_hi"]) as msghip,
              tc.tile_pool(name="msg_lo2", bufs=BUFS["msg_lo2"]) as msglo2p,
              tc.tile_pool(name="idxp", bufs=BUFS["idxp"]) as idxp,
              tc.tile_pool(name="spp", bufs=BUFS["spp"]) as spp,
              tc.tile_pool(name="blkp", bufs=BUFS["blkp"]) as blkp,
              tc.tile_pool(name="psh", bufs=BUFS["psh"], space="PSUM") as psum_h,
              tc.tile_pool(name="pstr", bufs=BUFS["pstr"], space="PSUM") as psum_tr,
              tc.tile_pool(name="psout", bufs=BUFS["psout"], space="PSUM") as psum_out):

            iota_i = constp.tile([P, P], dt.int32)
            nc.gpsimd.iota(iota_i[:], pattern=[[1, P]], base=0, channel_multiplier=0)
            iota_f = constp.tile([P, P], dt.bfloat16)
            nc.vector.tensor_copy(iota_f[:], iota_i[:])
            ident = constp.tile([P, P], dt.bfloat16)
            make_identity(nc, ident[:])
            w2_sb = constp.tile([P, 2, P], dt.bfloat16)
            nc.sync.dma_start(out=w2_sb[:], in_=w2_d[:])
            rowm1_sb = metap.tile([P, NCH1], dt.float32)
            nc.sync.dma_start(out=rowm1_sb[:], in_=rowm1_d[:])
            nvm1_sb = metap.tile([P, NCH1], dt.float32)
            nc.sync.dma_start(out=nvm1_sb[:], in_=nvm1_d[:])
            # hop-2 meta tiles are loaded at the end of hop 1 (see below) so
            # the first gathers aren't queued behind their DMA at startup.
            rowm2_sb = metap.tile([P, NCH2], dt.float32)
            nvm2_sb = metap.tile([P, NCH2], dt.float32)

            slab_cache = {}
            idx_cache = {}
            IDX_SLAB = 4096   # tokens per idx load: 512B/partition rows, so
                              # the idx DMA avoids the sub-512B descriptor
                              # latency penalty (GATHER_SLAB must divide it)

            def get_chunk(stream, src_ap, pool, T, gpos):
                tile_obj, s_cur = slab_cache.get(stream, (None, -1))
                s, j = divmod(gpos, GATHER_SLAB // P)
                if s != s_cur:
                    off = s * GATHER_SLAB
                    g = min(GATHER_SLAB, T - off)
                    si, so = divmod(off, IDX_SLAB)
                    it, si_cur = idx_cache.get(stream, (None, -1))
                    if si != si_cur:
                        gi = min(IDX_SLAB, T - si * IDX_SLAB)
                        it = idxp.tile([P, gi // 16], dt.int16, tag="idx")
                        nc.sync.dma_start(
                            out=it[:],
                            in_=idx_d[stream][:, si * IDX_SLAB // 16:
                                              (si * IDX_SLAB + gi) // 16])
                        idx_cache[stream] = (it, si)
                    mt = pool.tile([P, g // P, TOKC], dt.bfloat16, tag="m" + stream)
                    nc.gpsimd.dma_gather(
                        out_ap=mt[:], in_ap=src_ap,
                        idxs_ap=it[:, so // 16:(so + g) // 16],
                        num_idxs=g, num_idxs_reg=g,
                        elem_size=TOKC, single_packet=False)
                    slab_cache[stream] = (mt, s)
                    tile_obj = mt
                return tile_obj, j

            def build_sp(rowm_sb, nvm_sb, c):
                sp = spp.tile([P, P], dt.bfloat16, tag="sp")
                nc.vector.tensor_scalar(
                    sp[:], iota_f[:],
                    rowm_sb[:, c:c + 1], nvm_sb[:, c:c + 1],
                    mybir.AluOpType.is_equal, mybir.AluOpType.mult)
                return sp

            # ---- hop 1: h1[half] = (D^-1 A) h0;  outA = h1 @ W[0] and
            # z2 = h1 @ W[1] (W commutes with A: out = h1 W0 + A (h1 W1),
            # so hop 2 needs no W stage at all). ----
            for p in range(NPOS1):
                hp = psum_h.tile([P, 2, P], dt.float32, tag="hp")
                steps = [('lo', j, int(cco1[p]) + j - int(csL1[p]))
                         for j in range(int(csL1[p]), int(ceL1[p]) + 1)]
                if t_hi1[p] > 0:
                    steps += [('hi', j,
                               int(cco1[p] + ncL1[p]) + j - int(csH1[p]))
                              for j in range(int(csH1[p]), int(ceH1[p]) + 1)]
                for i, (stream, j, col) in enumerate(steps):
                    if stream == 'lo':
                        mt, jj = get_chunk('lo', x2[0:LO_LIMIT, :], msglop,
                                           TLO1, j)
                    else:
                        mt, jj = get_chunk('hi', x2[HI_BASE:NNP, :], msghip,
                                           THI1, j)
                    sp = build_sp(rowm1_sb, nvm1_sb, col)
                    nc.tensor.matmul(hp[:], sp[:], mt[:, jj, :],
                                     start=(i == 0), stop=(i == len(steps) - 1))
                h_sb = blkp.tile([P, 2, P], dt.bfloat16, tag="h_sb")
                nc.scalar.copy(h_sb[:], hp[:])
                tr = psum_tr.tile([P, 2, P], dt.bfloat16, tag="tr")
                nc.tensor.transpose(tr[:, 0, :], h_sb[:, 0, :], ident[:])
                nc.tensor.transpose(tr[:, 1, :], h_sb[:, 1, :], ident[:])
                trs = blkp.tile([P, 2, P], dt.bfloat16, tag="trs")
                nc.vector.tensor_copy(trs[:], tr[:])
                for k, dest in ((1, z2), (0, outA)):
                    op = psum_out.tile([P, 2, P], dt.float32, tag="op")
                    nc.tensor.matmul(op[:, 0, :], trs[:, 0, :], w2_sb[:, k, :],
                                     start=True, stop=True)
                    nc.tensor.matmul(op[:, 1, :], trs[:, 1, :], w2_sb[:, k, :],
                                     start=True, stop=True)
                    ob = blkp.tile([P, 2, P], dt.bfloat16, tag="ob")
                    if k == 1:
                        nc.scalar.copy(ob[:], op[:])
                    else:
                        nc.vector.tensor_copy(ob[:], op[:])
                    nc.sync.dma_start(out=dest[p * P:(p + 1) * P, :], in_=ob[:])

            # ---- hop 2: outB = (D^-1 A)|src-half z2 (final partial) ----
            # Boundary-sharing stream: chunk ranges [cs2[p], ce2[p]] overlap
            # between adjacent positions; each position has its own nv=0
            # masked meta column for a shared chunk.
            # The barrier orders hop-2's z2 gathers after hop-1's z2 writes
            # (DRAM RAW is not tracked at tile granularity).
            nc.sync.dma_start(out=rowm2_sb[:], in_=rowm2_d[:])
            nc.sync.dma_start(out=nvm2_sb[:], in_=nvm2_d[:])
            tc.strict_bb_all_engine_barrier()
            for p in range(NB):
                hp = psum_h.tile([P, 2, P], dt.float32, tag="hp")
                for j in range(int(cs2[p]), int(ce2[p]) + 1):
                    mt, jj = get_chunk('lo2', z2[0:HALF_ROWS, :], msglo2p, T2, j)
                    sp = build_sp(rowm2_sb, nvm2_sb,
                                  int(cco2[p]) + j - int(cs2[p]))
                    nc.tensor.matmul(hp[:], sp[:], mt[:, jj, :],
                                     start=(j == int(cs2[p])),
                                     stop=(j == int(ce2[p])))
                ob = blkp.tile([P, 2, P], dt.bfloat16, tag="ob")
                nc.scalar.copy(ob[:], hp[:])
                nc.sync.dma_start(out=outB[p * P:(p + 1) * P, :], in_=ob[:])

    nc.compile()
    return nc


# ---------------- entry point ----------------

def kernel(x, edge_index, edge_vals, W_f, W_b, bias):
    x = np.asarray(x, dtype=np.float32)
    edge_index = np.asarray(edge_index)
    edge_vals = np.asarray(edge_vals, dtype=np.float32)
    W_f = np.asarray(W_f, dtype=np.float32)
    W_b = np.asarray(W_b, dtype=np.float32)
    bias = np.asarray(bias, dtype=np.float32)

    rows = edge_index[0].astype(np.int64)
    cols = edge_index[1].astype(np.int64)
    deg = np.zeros(N_NODES, np.float32)
    np.add.at(deg, rows, edge_vals)
    deg += np.float32(1e-8)
    nv = (edge_vals / deg[rows]).astype(np.float32)

    halves = [np.arange(0, NPOS1), np.arange(NPOS1, NB)]
    v1 = []   # hop-1 variants: (pos, dstloc, src, nv, order)
    v2 = []   # hop-2 variants: (pos, dstloc, srcloc, nv, order, inv1)
    for d in range(2):
        dst, src = (rows, cols) if d == 0 else (cols, rows)
        dblk = dst >> 7
        dloc = dst & (P - 1)
        sblk = src >> 7
        tot1 = np.bincount(dblk, minlength=NB)
        tot2 = np.bincount(dblk, weights=(sblk >= NPOS1).astype(np.float64),
                           minlength=NB)
        for h in range(2):
            sel = (dblk >= NPOS1) == (h == 1)
            order1, inv1 = _pack_positions(tot1, halves[h], NPOS1)
            v1.append((inv1[dblk[sel]], dloc[sel], src[sel], nv[sel], order1))
            sel2 = (sblk >= NPOS1) == (h == 1)
            t2 = tot2 if h == 1 else (tot1 - tot2)
            order2, inv2 = _pack_positions(t2, np.arange(NB), NB)
            srcloc = inv1[sblk[sel2]] * P + (src[sel2] & (P - 1))
            v2.append((inv2[dblk[sel2]], dloc[sel2], srcloc, nv[sel2],
                       order2, inv1))

    c1 = [_classify(pos, src, NPOS1) for (pos, _, src, _, _) in v1]
    t_lo1, t_hi1, f2l1 = _choose_split(c1, NPOS1)
    mt2 = np.maximum.reduce([np.bincount(pos, minlength=NB)
                             for (pos, _, src, _, _, _) in v2])
    mt2 = np.maximum(mt2, 1)

    s1 = [_build_stream1(pos, dl, src, nvv, t_lo1, t_hi1, f2l1[i])
          for i, (pos, dl, src, nvv, _) in enumerate(v1)]
    s2 = [_build_stream2(pos, dl, src, nvv, mt2)
          for (pos, dl, src, nvv, _, _) in v2]

    key = (t_lo1.tobytes(), t_hi1.tobytes(), mt2.tobytes())
    if key not in _prog_cache:
        _prog_cache.clear()
        _prog_cache[key] = _build_program(t_lo1, t_hi1, mt2)
    nc = _prog_cache[key]

    in_maps = []
    for core in range(8):
        d, g, h = core >> 2, (core >> 1) & 1, core & 1
        vi = d * 2 + h
        Wd = W_f if d == 0 else W_b
        x2 = np.zeros((NNP, TOKC), bf16)
        x2[:N_NODES] = x[4 * g:4 * g + 4].transpose(1, 0, 2).reshape(
            N_NODES, TOKC).astype(bf16)
        w2 = np.zeros((P, 2, P), bf16)
        for k in range(2):
            for a in range(2):
                w2[C * a:C * a + C, k, C * a:C * a + C] = Wd[k].astype(bf16)
        in_maps.append({
            "x2": x2, "w2": w2,
            "idx_lo": s1[vi][0], "idx_hi": s1[vi][1],
            "rowm1": s1[vi][2], "nvm1": s1[vi][3],
            "idx2": s2[vi][0],
            "rowm2": s2[vi][1], "nvm2": s2[vi][2],
        })

    results = run_bass_kernel_spmd(nc, in_maps, list(range(8))).results

    out = np.empty((B, N_NODES, C), np.float32)
    for g in range(2):
        acc = np.zeros((NNP, TOKC), np.float32)
        for d in range(2):
            for h in range(2):
                vi = d * 2 + h
                r = results[(d << 2) | (g << 1) | h]
                order2 = v2[vi][4]
                inv2 = np.argsort(order2)
                acc += np.asarray(r["outB"]).astype(np.float32).reshape(
                    NB, P, TOKC)[inv2].reshape(NNP, TOKC)
                order1 = v1[vi][4]
                nreal = halves[h].size
                oa = np.asarray(r["outA"]).astype(np.float32).reshape(
                    NPOS1, P, TOKC)[:nreal]
                accb = acc.reshape(NB, P, TOKC)
                accb[order1[:nreal]] += oa
        for bl in range(4):
            out[4 * g + bl] = acc[:N_NODES, C * bl:C * bl + C]
    out += bias.reshape(1, 1, C)
    return out
